# revision 12
# baseline (speedup 1.0000x reference)
"""MoE gate (router) kernel for Trainium2 — v2: big-line DMA layout.

Computes, for hidden_states [T, H] and gate weight [E, H]:
    logits = hidden_states @ weight.T          # [T, E]
    probs  = softmax(logits, axis=-1)
    topk_weight, topk_idx = top_k(probs, 8)    # normalized over the top-8
    row_idx = arange(T*8).reshape(8, T).T

Strategy (8 NeuronCores, data parallel over tokens):
  - fp32 accuracy from fp16 hi/lo splits (host-side, same DMA bytes as f32):
    hs = hi + lo/2^11, 64*w = whi + wlo/2^11.  Three fp16 matmul terms:
        psum[:, 0:512]   += hshi . [whi | wlo]
        psum[:, 256:512] += hslo . whi          (same 2^11 scale as hi*lo)
    logits = 2^-6 * psum[:, 0:256] + 2^-17 * psum[:, 256:512]
  - v2/v3 DMA layout: the baseline DMA'd [128, KC, 128-token] tiles whose
    256-byte contiguous lines pay the <512B half-bandwidth DMA penalty
    (NTFF: DMA 98% active at ~187GB/s, 57us of PE idle).  Now hs is
    pre-arranged host-side as [p=128][group][ko][512 tokens] so each
    k-chunk DMA moves 8KB-contiguous per-partition lines at full
    bandwidth (NTFF: ~304GB/s), with hi on the SP queue and lo on the
    Activation queue across both HWDGE rings.  The gate weight loads in
    NKC chunks on the gpsimd (SWDGE) queue so the first matmul waits on
    ~1MB instead of the whole 7.3MB.
  - Tokens are processed in four 512-token groups; each group accumulates
    4 x 128-token subtiles in 4 PSUM banks across all 56 k-tiles, so two
    groups are in flight and the top-k evacuation of group g overlaps the
    matmuls of group g+1 instead of stalling the PE at group boundaries.
  - DVE max/max_index give top-8 values+indices; softmax over the full
    256 experts + top-k renorm reduces to a softmax over the top-8 logits.
"""

import numpy as np

TOP_K = 8
NUM_EXPERTS = 256
HIDDEN = 7168
NUM_TOKENS = 16384
N_CORES = 8
T_LOC = NUM_TOKENS // N_CORES

W_SCALE = 64.0       # weight pre-scale so fp16(64*w) stays normal-range
LO_SCALE = 2048.0    # 2^11: lo parts carry the next 11 mantissa bits

P = 128
KT = HIDDEN // P     # 56 k-tiles along hidden dim
NG = 4               # token groups per core
T_GRP = T_LOC // NG  # 512 tokens per group
TS_GRP = T_GRP // P  # 4 subtiles per group = 4 PSUM banks (2 groups in flight)
KC = 8               # k-tiles per DMA chunk (8KB/partition lines)
NKC = KT // KC       # 7 chunks

_NC_CACHE = {}


def build_gate_nc(t_loc=T_LOC, h=HIDDEN, e=NUM_EXPERTS, repeat=1):
    import concourse.mybir as mybir
    import concourse.tile as tile
    from concourse import bacc

    f32 = mybir.dt.float32
    fp16 = mybir.dt.float16

    nc = bacc.Bacc("TRN2", target_bir_lowering=False)
    # [p][group][ko][t]: per-partition contiguous KC*512 fp16 = 8KB chunks
    hsT_hi = nc.dram_tensor("hsT_hi", [P, NG * KT * T_GRP], fp16, kind="ExternalInput")
    hsT_lo = nc.dram_tensor("hsT_lo", [P, NG * KT * T_GRP], fp16, kind="ExternalInput")
    # [p][ko][0:256]=fp16(64*wT), [p][ko][256:512]=fp16((64*wT - hi) * 2^11)
    wT_cat = nc.dram_tensor("wT_cat", [P, KT * 2 * e], fp16, kind="ExternalInput")
    # outputs leave in on-chip staging order [p][ts][k]; host reorders to
    # [ts*128+p, k] (free) so the DMA moves one contiguous 512B line per
    # partition instead of 2048 x 32B descriptors
    idx_out = nc.dram_tensor(
        "topk_idx", [P, (t_loc // P) * TOP_K], mybir.dt.int32, kind="ExternalOutput"
    )
    w_out = nc.dram_tensor("topk_w", [P, (t_loc // P) * TOP_K], f32, kind="ExternalOutput")

    TS = t_loc // P  # 16 subtiles total

    with tile.TileContext(nc) as tc:
        with (
            tc.tile_pool(name="wpool", bufs=1) as wpool,
            tc.tile_pool(name="hpool", bufs=6) as hpool,
            tc.tile_pool(name="lpool", bufs=3) as lpool,
            tc.tile_pool(name="spool", bufs=4) as spool,
            tc.tile_pool(name="psum", bufs=8, space="PSUM") as psum_pool,
        ):
            # output staging: results accumulate here and leave as two large
            # descriptor DMAs at the end
            stage_idx = wpool.tile([P, TS, TOP_K], mybir.dt.int32, tag="sidx")
            stage_wv = wpool.tile([P, TS, TOP_K], f32, tag="swv")
            # gate weight: resident in SBUF, loaded in NKC chunks so the first
            # matmul waits on ~1MB, not 7.3MB.  Chunks 1/2 ride the SP/Act
            # queues ahead of the hs stream (issued first in program order);
            # the rest go to the gpsimd SWDGE queue, which is otherwise idle —
            # this removes the early LDWEIGHTS stalls seen when all weight
            # chunks trickled through the single SWDGE queue.
            wt = wpool.tile([P, KT, 2 * e], fp16, tag="wt")
            wt_view = wT_cat[:, :].rearrange("p (ko e) -> p ko e", ko=KT)
            wt_engines = {1: nc.sync, 2: nc.scalar}
            for kc in range(NKC):
                kslc = slice(kc * KC, (kc + 1) * KC)
                eng = wt_engines.get(kc, nc.gpsimd)
                eng.dma_start(wt[:, kslc, :], wt_view[:, kslc, :])
            for rep in range(repeat):
                for grp in range(NG):
                    pts = []
                    for ts_i in range(TS_GRP):
                        pts.append(
                            psum_pool.tile(
                                [P, 2 * e], f32, tag="pt",
                                name=f"pt{rep}_{grp}_{ts_i}",
                            )
                        )
                    for kc in range(NKC):
                        base = grp * (KT * T_GRP) + kc * (KC * T_GRP)
                        hhi = hpool.tile(
                            [P, KC, T_GRP], fp16, tag="hs",
                            name=f"hshi{rep}_{grp}_{kc}",
                        )
                        nc.sync.dma_start(
                            hhi,
                            hsT_hi[:, base : base + KC * T_GRP].rearrange(
                                "p (ko t) -> p ko t", ko=KC
                            ),
                        )
                        hlo = hpool.tile(
                            [P, KC, T_GRP], fp16, tag="hs",
                            name=f"hslo{rep}_{grp}_{kc}",
                        )
                        nc.scalar.dma_start(
                            hlo,
                            hsT_lo[:, base : base + KC * T_GRP].rearrange(
                                "p (ko t) -> p ko t", ko=KC
                            ),
                        )
                        for ki in range(KC):
                            k = kc * KC + ki
                            for ts_i in range(TS_GRP):
                                pt = pts[ts_i]
                                tslc = slice(ts_i * P, (ts_i + 1) * P)
                                # psum[:, 0:2e] += hshi . [whi | wlo]
                                nc.tensor.matmul(
                                    pt,
                                    hhi[:, ki, tslc],
                                    wt[:, k, :],
                                    start=(k == 0),
                                    stop=False,
                                )
                                # psum[:, e:2e] += hslo . whi  (2^11 scale)
                                nc.tensor.matmul(
                                    pt[:, e:],
                                    hlo[:, ki, tslc],
                                    wt[:, k, :e],
                                    start=False,
                                    stop=(k == KT - 1),
                                )
                    for ts_i in range(TS_GRP):
                        pt = pts[ts_i]
                        g_ts = grp * TS_GRP + ts_i
                        # logits = 2^-6 * psum_hi + 2^-17 * psum_cross
                        cross = lpool.tile([P, e], f32, tag="cross")
                        nc.vector.tensor_scalar_mul(
                            cross, pt[:, e:], 1.0 / (64.0 * 2048.0)
                        )
                        logits = lpool.tile([P, e], f32, tag="logits")
                        nc.vector.tensor_scalar(
                            logits,
                            pt[:, :e],
                            1.0 / 64.0,
                            None,
                            mybir.AluOpType.mult,
                        )
                        nc.vector.tensor_add(logits, logits, cross)
                        mx = spool.tile([P, TOP_K], f32, tag="mx")
                        nc.vector.max(out=mx, in_=logits)
                        idx_u = spool.tile([P, TOP_K], mybir.dt.uint32, tag="idxu")
                        nc.vector.max_index(idx_u, mx, logits)
                        nc.vector.tensor_copy(stage_idx[:, g_ts, :], idx_u)
                        # normalized top-k softmax: exp(v - v_max) / sum
                        nm = spool.tile([P, 1], f32, tag="nm")
                        nc.vector.tensor_scalar_mul(nm, mx[:, 0:1], -1.0)
                        ev = spool.tile([P, TOP_K], f32, tag="ev")
                        sm = spool.tile([P, 1], f32, tag="sm")
                        nc.scalar.activation(
                            ev,
                            mx,
                            mybir.ActivationFunctionType.Exp,
                            bias=nm,
                            scale=1.0,
                            accum_out=sm,
                        )
                        rc = spool.tile([P, 1], f32, tag="rc")
                        nc.vector.reciprocal(rc, sm)
                        nc.vector.tensor_scalar_mul(stage_wv[:, g_ts, :], ev, rc)
            nc.sync.dma_start(
                idx_out[:, :].rearrange("p (ts k) -> p ts k", k=TOP_K), stage_idx
            )
            nc.scalar.dma_start(
                w_out[:, :].rearrange("p (ts k) -> p ts k", k=TOP_K), stage_wv
            )
    nc.compile()
    return nc


def _get_nc():
    key = (T_LOC, HIDDEN, NUM_EXPERTS)
    if key not in _NC_CACHE:
        _NC_CACHE[key] = build_gate_nc(*key)
    return _NC_CACHE[key]


def _split_fp16(x, pre_scale=1.0):
    """x (f32) -> (hi, lo) fp16 with hi + lo/2^11 ~= pre_scale*x."""
    xs = x * np.float32(pre_scale) if pre_scale != 1.0 else x
    hi = xs.astype(np.float16)
    lo = ((xs - hi.astype(np.float32)) * np.float32(LO_SCALE)).astype(np.float16)
    return hi, lo


def _pack_hs(hs_part):
    """[t_loc, H] f32 -> [128, 4*56*512] fp16 hi/lo in [p][group][ko][t] order."""
    hsT = np.ascontiguousarray(hs_part.T)  # [H, t_loc]
    hi, lo = _split_fp16(hsT)
    out = []
    for x in (hi, lo):
        x4 = x.reshape(KT, P, NG, T_GRP)          # [ko][p][group][t]
        x4 = np.ascontiguousarray(x4.transpose(1, 2, 0, 3))  # [p][group][ko][t]
        out.append(x4.reshape(P, NG * KT * T_GRP))
    return out


def _prep_inputs(hs, w):
    wT = np.ascontiguousarray(w.T)  # [H, E]
    w_hi, w_lo = _split_fp16(wT, W_SCALE)
    w_cat = np.concatenate([w_hi, w_lo], axis=1)          # [H, 2E]
    w3 = w_cat.reshape(KT, P, 2 * NUM_EXPERTS)            # [ko][p][2e]
    w3 = np.ascontiguousarray(w3.transpose(1, 0, 2))      # [p][ko][2e]
    wT_packed = w3.reshape(P, KT * 2 * NUM_EXPERTS)
    in_maps = []
    for c in range(N_CORES):
        hs_hi, hs_lo = _pack_hs(hs[c * T_LOC : (c + 1) * T_LOC])
        in_maps.append({"hsT_hi": hs_hi, "hsT_lo": hs_lo, "wT_cat": wT_packed})
    return in_maps


_FN_CACHE = {}


def _make_runner(nc):
    """Compile a reusable 8-core PJRT callable (same lowering path as
    run_bass_kernel_spmd under axon, but cached so repeat kernel() calls
    skip re-tracing/compiling)."""
    import jax
    import concourse.mybir as mybir
    from concourse import bass2jax
    from jax.sharding import Mesh, NamedSharding, PartitionSpec
    from jax.experimental.shard_map import shard_map

    bass2jax.install_neuronx_cc_hook()
    partition_name = nc.partition_id_tensor.name if nc.partition_id_tensor else None
    in_names, out_names, out_avals, zero_shapes = [], [], [], []
    for alloc in nc.m.functions[0].allocations:
        if not isinstance(alloc, mybir.MemoryLocationSet):
            continue
        name = alloc.memorylocations[0].name
        if alloc.kind == "ExternalInput":
            if name != partition_name:
                in_names.append(name)
        elif alloc.kind == "ExternalOutput":
            shape = tuple(alloc.tensor_shape)
            dtype = mybir.dt.np(alloc.dtype)
            out_names.append(name)
            out_avals.append(jax.core.ShapedArray(shape, dtype))
            zero_shapes.append((shape, dtype))
    n_params = len(in_names)
    n_outs = len(out_avals)
    all_in_names = list(in_names) + list(out_names)
    if partition_name is not None:
        all_in_names.append(partition_name)

    def _body(*args):
        operands = list(args)
        if partition_name is not None:
            operands.append(bass2jax.partition_id_tensor())
        outs = bass2jax._bass_exec_p.bind(
            *operands,
            out_avals=tuple(out_avals),
            in_names=tuple(all_in_names),
            out_names=tuple(out_names),
            lowering_input_output_aliases=(),
            sim_require_finite=True,
            sim_require_nnan=True,
            nc=nc,
        )
        return tuple(outs)

    devices = jax.devices()[:N_CORES]
    mesh = Mesh(np.asarray(devices), ("core",))
    in_specs = (PartitionSpec("core"),) * (n_params + n_outs)
    out_specs = (PartitionSpec("core"),) * len(out_names)
    donate = tuple(range(n_params, n_params + n_outs))
    fn = jax.jit(
        shard_map(
            _body, mesh=mesh, in_specs=in_specs, out_specs=out_specs, check_rep=False
        ),
        donate_argnums=donate,
        keep_unused=True,
    )
    sharding = NamedSharding(mesh, PartitionSpec("core"))

    def run(in_maps):
        concat_in = [
            np.concatenate(
                [np.asarray(in_maps[c][nm]) for c in range(N_CORES)], axis=0
            )
            for nm in in_names
        ]
        zeros = [
            np.zeros((N_CORES * s[0], *s[1:]), dt) for s, dt in zero_shapes
        ]
        dev_in = [jax.device_put(x, sharding) for x in concat_in]
        out_arrs = fn(*dev_in, *zeros)
        return [
            {
                nm: np.asarray(out_arrs[i]).reshape(
                    N_CORES, *out_avals[i].shape
                )[c]
                for i, nm in enumerate(out_names)
            }
            for c in range(N_CORES)
        ]

    return run


def kernel(hidden_states, weight):
    hs = np.asarray(hidden_states, dtype=np.float32)
    w = np.asarray(weight, dtype=np.float32)
    assert hs.shape == (NUM_TOKENS, HIDDEN), hs.shape
    assert w.shape == (NUM_EXPERTS, HIDDEN), w.shape

    in_maps = _prep_inputs(hs, w)
    nc = _get_nc()
    try:
        if "run" not in _FN_CACHE:
            _FN_CACHE["run"] = _make_runner(nc)
        results = _FN_CACHE["run"](in_maps)
    except Exception:
        # fall back to the stock path if the cached-runner path breaks
        from concourse.bass_utils import run_bass_kernel_spmd

        results = run_bass_kernel_spmd(
            nc, in_maps, core_ids=list(range(N_CORES))
        ).results

    def _unstage(x):
        # [P, TS*K] staging order -> [t_loc, K] (token = ts*128 + p)
        ts_n = T_LOC // P
        return x.reshape(P, ts_n, TOP_K).transpose(1, 0, 2).reshape(T_LOC, TOP_K)

    topk_idx = np.concatenate([_unstage(r["topk_idx"]) for r in results], axis=0)
    topk_w = np.concatenate([_unstage(r["topk_w"]) for r in results], axis=0)
    row_idx = (
        np.arange(NUM_TOKENS * TOP_K, dtype=np.int32).reshape(TOP_K, NUM_TOKENS).T
    )
    return (
        topk_idx.astype(np.int32),
        topk_w.astype(np.float32),
        row_idx,
    )


# revision 13
# speedup vs baseline: 1.1836x; 1.1836x over previous
"""MoE gate (router) kernel for Trainium2 — v2: big-line DMA layout.

Computes, for hidden_states [T, H] and gate weight [E, H]:
    logits = hidden_states @ weight.T          # [T, E]
    probs  = softmax(logits, axis=-1)
    topk_weight, topk_idx = top_k(probs, 8)    # normalized over the top-8
    row_idx = arange(T*8).reshape(8, T).T

Strategy (8 NeuronCores, data parallel over tokens):
  - fp32 accuracy from fp16 hi/lo splits (host-side, same DMA bytes as f32):
    hs = hi + lo/2^11, 64*w = whi + wlo/2^11.  Three fp16 matmul terms:
        psum[:, 0:512]   += hshi . [whi | wlo]
        psum[:, 256:512] += hslo . whi          (same 2^11 scale as hi*lo)
    logits = 2^-6 * psum[:, 0:256] + 2^-17 * psum[:, 256:512]
  - v2/v3 DMA layout: the baseline DMA'd [128, KC, 128-token] tiles whose
    256-byte contiguous lines pay the <512B half-bandwidth DMA penalty
    (NTFF: DMA 98% active at ~187GB/s, 57us of PE idle).  Now hs is
    pre-arranged host-side as [p=128][group][ko][512 tokens] so each
    k-chunk DMA moves 8KB-contiguous per-partition lines at full
    bandwidth (NTFF: ~304GB/s), with hi on the SP queue and lo on the
    Activation queue across both HWDGE rings.  The gate weight loads in
    NKC chunks on the gpsimd (SWDGE) queue so the first matmul waits on
    ~1MB instead of the whole 7.3MB.
  - Tokens are processed in four 512-token groups; each group accumulates
    4 x 128-token subtiles in 4 PSUM banks across all 56 k-tiles, so two
    groups are in flight and the top-k evacuation of group g overlaps the
    matmuls of group g+1 instead of stalling the PE at group boundaries.
  - DVE max/max_index give top-8 values+indices; softmax over the full
    256 experts + top-k renorm reduces to a softmax over the top-8 logits.
"""

import numpy as np

TOP_K = 8
NUM_EXPERTS = 256
HIDDEN = 7168
NUM_TOKENS = 16384
N_CORES = 8
T_LOC = NUM_TOKENS // N_CORES

W_SCALE = 64.0       # weight pre-scale so fp16(64*w) stays normal-range
LO_SCALE = 2048.0    # 2^11: lo parts carry the next 11 mantissa bits

P = 128
KT = HIDDEN // P     # 56 k-tiles along hidden dim
NG = 4               # token groups per core
T_GRP = T_LOC // NG  # 512 tokens per group
TS_GRP = T_GRP // P  # 4 subtiles per group = 4 PSUM banks (2 groups in flight)
KC = 8               # k-tiles per DMA chunk (8KB/partition lines)
NKC = KT // KC       # 7 chunks

_NC_CACHE = {}


def build_gate_nc(t_loc=T_LOC, h=HIDDEN, e=NUM_EXPERTS, repeat=1):
    import concourse.mybir as mybir
    import concourse.tile as tile
    from concourse import bacc

    f32 = mybir.dt.float32
    fp16 = mybir.dt.float16

    nc = bacc.Bacc("TRN2", target_bir_lowering=False)
    # [p][group][ko][t]: per-partition contiguous KC*512 fp16 = 8KB chunks
    hsT_hi = nc.dram_tensor("hsT_hi", [P, NG * KT * T_GRP], fp16, kind="ExternalInput")
    hsT_lo = nc.dram_tensor("hsT_lo", [P, NG * KT * T_GRP], fp16, kind="ExternalInput")
    # [p][ko][0:256]=fp16(64*wT), [p][ko][256:512]=fp16((64*wT - hi) * 2^11)
    wT_cat = nc.dram_tensor("wT_cat", [P, KT * 2 * e], fp16, kind="ExternalInput")
    # outputs leave in on-chip staging order [p][ts][k]; host reorders to
    # [ts*128+p, k] (free) so the DMA moves one contiguous 512B line per
    # partition instead of 2048 x 32B descriptors
    idx_out = nc.dram_tensor(
        "topk_idx", [P, (t_loc // P) * TOP_K], mybir.dt.int32, kind="ExternalOutput"
    )
    w_out = nc.dram_tensor("topk_w", [P, (t_loc // P) * TOP_K], f32, kind="ExternalOutput")

    TS = t_loc // P  # 16 subtiles total

    with tile.TileContext(nc) as tc:
        with (
            tc.tile_pool(name="wpool", bufs=1) as wpool,
            tc.tile_pool(name="hpool", bufs=6) as hpool,
            tc.tile_pool(name="lpool", bufs=3) as lpool,
            tc.tile_pool(name="spool", bufs=4) as spool,
            tc.tile_pool(name="psum", bufs=8, space="PSUM") as psum_pool,
        ):
            # output staging: results accumulate here and leave as two large
            # descriptor DMAs at the end
            stage_idx = wpool.tile([P, TS, TOP_K], mybir.dt.int32, tag="sidx")
            stage_wv = wpool.tile([P, TS, TOP_K], f32, tag="swv")
            # gate weight: resident in SBUF, loaded in NKC chunks on the
            # otherwise-idle gpsimd (SWDGE) queue so the first matmul waits
            # on ~1MB, not 7.3MB.  (Routing early weight chunks via SP/Act
            # instead was tried and costs 70us/device: it delays the hs
            # stream those queues carry.)
            wt = wpool.tile([P, KT, 2 * e], fp16, tag="wt")
            wt_view = wT_cat[:, :].rearrange("p (ko e) -> p ko e", ko=KT)
            for kc in range(NKC):
                kslc = slice(kc * KC, (kc + 1) * KC)
                nc.gpsimd.dma_start(wt[:, kslc, :], wt_view[:, kslc, :])
            for rep in range(repeat):
                for grp in range(NG):
                    pts = []
                    for ts_i in range(TS_GRP):
                        pts.append(
                            psum_pool.tile(
                                [P, 2 * e], f32, tag="pt",
                                name=f"pt{rep}_{grp}_{ts_i}",
                            )
                        )
                    for kc in range(NKC):
                        base = grp * (KT * T_GRP) + kc * (KC * T_GRP)
                        hhi = hpool.tile(
                            [P, KC, T_GRP], fp16, tag="hs",
                            name=f"hshi{rep}_{grp}_{kc}",
                        )
                        nc.sync.dma_start(
                            hhi,
                            hsT_hi[:, base : base + KC * T_GRP].rearrange(
                                "p (ko t) -> p ko t", ko=KC
                            ),
                        )
                        hlo = hpool.tile(
                            [P, KC, T_GRP], fp16, tag="hs",
                            name=f"hslo{rep}_{grp}_{kc}",
                        )
                        nc.scalar.dma_start(
                            hlo,
                            hsT_lo[:, base : base + KC * T_GRP].rearrange(
                                "p (ko t) -> p ko t", ko=KC
                            ),
                        )
                        for ki in range(KC):
                            k = kc * KC + ki
                            for ts_i in range(TS_GRP):
                                pt = pts[ts_i]
                                tslc = slice(ts_i * P, (ts_i + 1) * P)
                                # psum[:, 0:2e] += hshi . [whi | wlo]
                                nc.tensor.matmul(
                                    pt,
                                    hhi[:, ki, tslc],
                                    wt[:, k, :],
                                    start=(k == 0),
                                    stop=False,
                                )
                                # psum[:, e:2e] += hslo . whi  (2^11 scale)
                                nc.tensor.matmul(
                                    pt[:, e:],
                                    hlo[:, ki, tslc],
                                    wt[:, k, :e],
                                    start=False,
                                    stop=(k == KT - 1),
                                )
                    for ts_i in range(TS_GRP):
                        pt = pts[ts_i]
                        g_ts = grp * TS_GRP + ts_i
                        # logits = 2^-6 * psum_hi + 2^-17 * psum_cross
                        cross = lpool.tile([P, e], f32, tag="cross")
                        nc.vector.tensor_scalar_mul(
                            cross, pt[:, e:], 1.0 / (64.0 * 2048.0)
                        )
                        logits = lpool.tile([P, e], f32, tag="logits")
                        nc.vector.tensor_scalar(
                            logits,
                            pt[:, :e],
                            1.0 / 64.0,
                            None,
                            mybir.AluOpType.mult,
                        )
                        nc.vector.tensor_add(logits, logits, cross)
                        mx = spool.tile([P, TOP_K], f32, tag="mx")
                        nc.vector.max(out=mx, in_=logits)
                        idx_u = spool.tile([P, TOP_K], mybir.dt.uint32, tag="idxu")
                        nc.vector.max_index(idx_u, mx, logits)
                        nc.vector.tensor_copy(stage_idx[:, g_ts, :], idx_u)
                        # normalized top-k softmax: exp(v - v_max) / sum
                        nm = spool.tile([P, 1], f32, tag="nm")
                        nc.vector.tensor_scalar_mul(nm, mx[:, 0:1], -1.0)
                        ev = spool.tile([P, TOP_K], f32, tag="ev")
                        sm = spool.tile([P, 1], f32, tag="sm")
                        nc.scalar.activation(
                            ev,
                            mx,
                            mybir.ActivationFunctionType.Exp,
                            bias=nm,
                            scale=1.0,
                            accum_out=sm,
                        )
                        rc = spool.tile([P, 1], f32, tag="rc")
                        nc.vector.reciprocal(rc, sm)
                        nc.vector.tensor_scalar_mul(stage_wv[:, g_ts, :], ev, rc)
            nc.sync.dma_start(
                idx_out[:, :].rearrange("p (ts k) -> p ts k", k=TOP_K), stage_idx
            )
            nc.scalar.dma_start(
                w_out[:, :].rearrange("p (ts k) -> p ts k", k=TOP_K), stage_wv
            )
    nc.compile()
    return nc


def _get_nc():
    key = (T_LOC, HIDDEN, NUM_EXPERTS)
    if key not in _NC_CACHE:
        _NC_CACHE[key] = build_gate_nc(*key)
    return _NC_CACHE[key]


def _split_fp16(x, pre_scale=1.0):
    """x (f32) -> (hi, lo) fp16 with hi + lo/2^11 ~= pre_scale*x."""
    xs = x * np.float32(pre_scale) if pre_scale != 1.0 else x
    hi = xs.astype(np.float16)
    lo = ((xs - hi.astype(np.float32)) * np.float32(LO_SCALE)).astype(np.float16)
    return hi, lo


def _pack_hs(hs_part):
    """[t_loc, H] f32 -> [128, 4*56*512] fp16 hi/lo in [p][group][ko][t] order."""
    hsT = np.ascontiguousarray(hs_part.T)  # [H, t_loc]
    hi, lo = _split_fp16(hsT)
    out = []
    for x in (hi, lo):
        x4 = x.reshape(KT, P, NG, T_GRP)          # [ko][p][group][t]
        x4 = np.ascontiguousarray(x4.transpose(1, 2, 0, 3))  # [p][group][ko][t]
        out.append(x4.reshape(P, NG * KT * T_GRP))
    return out


def _prep_inputs(hs, w):
    wT = np.ascontiguousarray(w.T)  # [H, E]
    w_hi, w_lo = _split_fp16(wT, W_SCALE)
    w_cat = np.concatenate([w_hi, w_lo], axis=1)          # [H, 2E]
    w3 = w_cat.reshape(KT, P, 2 * NUM_EXPERTS)            # [ko][p][2e]
    w3 = np.ascontiguousarray(w3.transpose(1, 0, 2))      # [p][ko][2e]
    wT_packed = w3.reshape(P, KT * 2 * NUM_EXPERTS)
    in_maps = []
    for c in range(N_CORES):
        hs_hi, hs_lo = _pack_hs(hs[c * T_LOC : (c + 1) * T_LOC])
        in_maps.append({"hsT_hi": hs_hi, "hsT_lo": hs_lo, "wT_cat": wT_packed})
    return in_maps


_FN_CACHE = {}


def _make_runner(nc):
    """Compile a reusable 8-core PJRT callable (same lowering path as
    run_bass_kernel_spmd under axon, but cached so repeat kernel() calls
    skip re-tracing/compiling)."""
    import jax
    import concourse.mybir as mybir
    from concourse import bass2jax
    from jax.sharding import Mesh, NamedSharding, PartitionSpec
    from jax.experimental.shard_map import shard_map

    bass2jax.install_neuronx_cc_hook()
    partition_name = nc.partition_id_tensor.name if nc.partition_id_tensor else None
    in_names, out_names, out_avals, zero_shapes = [], [], [], []
    for alloc in nc.m.functions[0].allocations:
        if not isinstance(alloc, mybir.MemoryLocationSet):
            continue
        name = alloc.memorylocations[0].name
        if alloc.kind == "ExternalInput":
            if name != partition_name:
                in_names.append(name)
        elif alloc.kind == "ExternalOutput":
            shape = tuple(alloc.tensor_shape)
            dtype = mybir.dt.np(alloc.dtype)
            out_names.append(name)
            out_avals.append(jax.core.ShapedArray(shape, dtype))
            zero_shapes.append((shape, dtype))
    n_params = len(in_names)
    n_outs = len(out_avals)
    all_in_names = list(in_names) + list(out_names)
    if partition_name is not None:
        all_in_names.append(partition_name)

    def _body(*args):
        operands = list(args)
        if partition_name is not None:
            operands.append(bass2jax.partition_id_tensor())
        outs = bass2jax._bass_exec_p.bind(
            *operands,
            out_avals=tuple(out_avals),
            in_names=tuple(all_in_names),
            out_names=tuple(out_names),
            lowering_input_output_aliases=(),
            sim_require_finite=True,
            sim_require_nnan=True,
            nc=nc,
        )
        return tuple(outs)

    devices = jax.devices()[:N_CORES]
    mesh = Mesh(np.asarray(devices), ("core",))
    in_specs = (PartitionSpec("core"),) * (n_params + n_outs)
    out_specs = (PartitionSpec("core"),) * len(out_names)
    donate = tuple(range(n_params, n_params + n_outs))
    fn = jax.jit(
        shard_map(
            _body, mesh=mesh, in_specs=in_specs, out_specs=out_specs, check_rep=False
        ),
        donate_argnums=donate,
        keep_unused=True,
    )
    sharding = NamedSharding(mesh, PartitionSpec("core"))

    def run(in_maps):
        concat_in = [
            np.concatenate(
                [np.asarray(in_maps[c][nm]) for c in range(N_CORES)], axis=0
            )
            for nm in in_names
        ]
        zeros = [
            np.zeros((N_CORES * s[0], *s[1:]), dt) for s, dt in zero_shapes
        ]
        dev_in = [jax.device_put(x, sharding) for x in concat_in]
        out_arrs = fn(*dev_in, *zeros)
        return [
            {
                nm: np.asarray(out_arrs[i]).reshape(
                    N_CORES, *out_avals[i].shape
                )[c]
                for i, nm in enumerate(out_names)
            }
            for c in range(N_CORES)
        ]

    return run


def kernel(hidden_states, weight):
    hs = np.asarray(hidden_states, dtype=np.float32)
    w = np.asarray(weight, dtype=np.float32)
    assert hs.shape == (NUM_TOKENS, HIDDEN), hs.shape
    assert w.shape == (NUM_EXPERTS, HIDDEN), w.shape

    in_maps = _prep_inputs(hs, w)
    nc = _get_nc()
    try:
        if "run" not in _FN_CACHE:
            _FN_CACHE["run"] = _make_runner(nc)
        results = _FN_CACHE["run"](in_maps)
    except Exception:
        # fall back to the stock path if the cached-runner path breaks
        from concourse.bass_utils import run_bass_kernel_spmd

        results = run_bass_kernel_spmd(
            nc, in_maps, core_ids=list(range(N_CORES))
        ).results

    def _unstage(x):
        # [P, TS*K] staging order -> [t_loc, K] (token = ts*128 + p)
        ts_n = T_LOC // P
        return x.reshape(P, ts_n, TOP_K).transpose(1, 0, 2).reshape(T_LOC, TOP_K)

    topk_idx = np.concatenate([_unstage(r["topk_idx"]) for r in results], axis=0)
    topk_w = np.concatenate([_unstage(r["topk_w"]) for r in results], axis=0)
    row_idx = (
        np.arange(NUM_TOKENS * TOP_K, dtype=np.int32).reshape(TOP_K, NUM_TOKENS).T
    )
    return (
        topk_idx.astype(np.int32),
        topk_w.astype(np.float32),
        row_idx,
    )


# revision 14
# speedup vs baseline: 1.2100x; 1.0223x over previous
"""MoE gate (router) kernel for Trainium2 — v2: big-line DMA layout.

Computes, for hidden_states [T, H] and gate weight [E, H]:
    logits = hidden_states @ weight.T          # [T, E]
    probs  = softmax(logits, axis=-1)
    topk_weight, topk_idx = top_k(probs, 8)    # normalized over the top-8
    row_idx = arange(T*8).reshape(8, T).T

Strategy (8 NeuronCores, data parallel over tokens):
  - fp32 accuracy from fp16 hi/lo splits (host-side, same DMA bytes as f32):
    hs = hi + lo/2^11, 64*w = whi + wlo/2^11.  Three fp16 matmul terms:
        psum[:, 0:512]   += hshi . [whi | wlo]
        psum[:, 256:512] += hslo . whi          (same 2^11 scale as hi*lo)
    logits = 2^-6 * psum[:, 0:256] + 2^-17 * psum[:, 256:512]
  - v2/v3 DMA layout: the baseline DMA'd [128, KC, 128-token] tiles whose
    256-byte contiguous lines pay the <512B half-bandwidth DMA penalty
    (NTFF: DMA 98% active at ~187GB/s, 57us of PE idle).  Now hs is
    pre-arranged host-side as [p=128][group][ko][512 tokens] so each
    k-chunk DMA moves 8KB-contiguous per-partition lines at full
    bandwidth (NTFF: ~304GB/s), with hi on the SP queue and lo on the
    Activation queue across both HWDGE rings.  The gate weight loads in
    NKC chunks on the gpsimd (SWDGE) queue so the first matmul waits on
    ~1MB instead of the whole 7.3MB.
  - Tokens are processed in four 512-token groups; each group accumulates
    4 x 128-token subtiles in 4 PSUM banks across all 56 k-tiles, so two
    groups are in flight and the top-k evacuation of group g overlaps the
    matmuls of group g+1 instead of stalling the PE at group boundaries.
  - DVE max/max_index give top-8 values+indices; softmax over the full
    256 experts + top-k renorm reduces to a softmax over the top-8 logits.
"""

import numpy as np

TOP_K = 8
NUM_EXPERTS = 256
HIDDEN = 7168
NUM_TOKENS = 16384
N_CORES = 8
T_LOC = NUM_TOKENS // N_CORES

W_SCALE = 64.0       # weight pre-scale so fp16(64*w) stays normal-range
LO_SCALE = 2048.0    # 2^11: lo parts carry the next 11 mantissa bits

P = 128
KT = HIDDEN // P     # 56 k-tiles along hidden dim
NG = 4               # token groups per core
T_GRP = T_LOC // NG  # 512 tokens per group
TS_GRP = T_GRP // P  # 4 subtiles per group = 4 PSUM banks (2 groups in flight)
KC = 8               # k-tiles per DMA chunk (8KB/partition lines)
NKC = KT // KC       # 7 chunks

_NC_CACHE = {}


def build_gate_nc(t_loc=T_LOC, h=HIDDEN, e=NUM_EXPERTS, repeat=1):
    import concourse.mybir as mybir
    import concourse.tile as tile
    from concourse import bacc

    f32 = mybir.dt.float32
    fp16 = mybir.dt.float16

    nc = bacc.Bacc("TRN2", target_bir_lowering=False)
    # [p][group][ko][t]: per-partition contiguous KC*512 fp16 = 8KB chunks
    hsT_hi = nc.dram_tensor("hsT_hi", [P, NG * KT * T_GRP], fp16, kind="ExternalInput")
    hsT_lo = nc.dram_tensor("hsT_lo", [P, NG * KT * T_GRP], fp16, kind="ExternalInput")
    # [p][ko][0:256]=fp16(64*wT), [p][ko][256:512]=fp16((64*wT - hi) * 2^11)
    wT_cat = nc.dram_tensor("wT_cat", [P, KT * 2 * e], fp16, kind="ExternalInput")
    # outputs leave in on-chip staging order [p][ts][k]; host reorders to
    # [ts*128+p, k] (free) so the DMA moves one contiguous 512B line per
    # partition instead of 2048 x 32B descriptors
    idx_out = nc.dram_tensor(
        "topk_idx", [P, (t_loc // P) * TOP_K], mybir.dt.int32, kind="ExternalOutput"
    )
    w_out = nc.dram_tensor("topk_w", [P, (t_loc // P) * TOP_K], f32, kind="ExternalOutput")

    TS = t_loc // P  # 16 subtiles total

    with tile.TileContext(nc) as tc:
        with (
            tc.tile_pool(name="wpool", bufs=1) as wpool,
            tc.tile_pool(name="hpool", bufs=6) as hpool,
            tc.tile_pool(name="lpool", bufs=3) as lpool,
            tc.tile_pool(name="spool", bufs=4) as spool,
            tc.tile_pool(name="psum", bufs=8, space="PSUM") as psum_pool,
        ):
            # output staging: results accumulate here and leave as two large
            # descriptor DMAs at the end
            stage_idx = wpool.tile([P, TS, TOP_K], mybir.dt.int32, tag="sidx")
            stage_wv = wpool.tile([P, TS, TOP_K], f32, tag="swv")
            # gate weight: resident in SBUF, loaded in NKC chunks on the
            # otherwise-idle gpsimd (SWDGE) queue so the first matmul waits
            # on ~1MB, not 7.3MB.  (Routing early weight chunks via SP/Act
            # instead was tried and costs 70us/device: it delays the hs
            # stream those queues carry.)
            wt = wpool.tile([P, KT, 2 * e], fp16, tag="wt")
            wt_view = wT_cat[:, :].rearrange("p (ko e) -> p ko e", ko=KT)
            KC_W = 4  # finer than the hs chunks: halves the early LDWEIGHTS
            for kc in range(KT // KC_W):  # stalls while SWDGE ramps
                kslc = slice(kc * KC_W, (kc + 1) * KC_W)
                nc.gpsimd.dma_start(wt[:, kslc, :], wt_view[:, kslc, :])
            for rep in range(repeat):
                for grp in range(NG):
                    pts = []
                    for ts_i in range(TS_GRP):
                        pts.append(
                            psum_pool.tile(
                                [P, 2 * e], f32, tag="pt",
                                name=f"pt{rep}_{grp}_{ts_i}",
                            )
                        )
                    for kc in range(NKC):
                        base = grp * (KT * T_GRP) + kc * (KC * T_GRP)
                        hhi = hpool.tile(
                            [P, KC, T_GRP], fp16, tag="hs",
                            name=f"hshi{rep}_{grp}_{kc}",
                        )
                        nc.sync.dma_start(
                            hhi,
                            hsT_hi[:, base : base + KC * T_GRP].rearrange(
                                "p (ko t) -> p ko t", ko=KC
                            ),
                        )
                        hlo = hpool.tile(
                            [P, KC, T_GRP], fp16, tag="hs",
                            name=f"hslo{rep}_{grp}_{kc}",
                        )
                        nc.scalar.dma_start(
                            hlo,
                            hsT_lo[:, base : base + KC * T_GRP].rearrange(
                                "p (ko t) -> p ko t", ko=KC
                            ),
                        )
                        for ki in range(KC):
                            k = kc * KC + ki
                            for ts_i in range(TS_GRP):
                                pt = pts[ts_i]
                                tslc = slice(ts_i * P, (ts_i + 1) * P)
                                # psum[:, 0:2e] += hshi . [whi | wlo]
                                nc.tensor.matmul(
                                    pt,
                                    hhi[:, ki, tslc],
                                    wt[:, k, :],
                                    start=(k == 0),
                                    stop=False,
                                )
                                # psum[:, e:2e] += hslo . whi  (2^11 scale)
                                nc.tensor.matmul(
                                    pt[:, e:],
                                    hlo[:, ki, tslc],
                                    wt[:, k, :e],
                                    start=False,
                                    stop=(k == KT - 1),
                                )
                    for ts_i in range(TS_GRP):
                        pt = pts[ts_i]
                        g_ts = grp * TS_GRP + ts_i
                        # logits = 2^-6 * psum_hi + 2^-17 * psum_cross
                        cross = lpool.tile([P, e], f32, tag="cross")
                        nc.vector.tensor_scalar_mul(
                            cross, pt[:, e:], 1.0 / (64.0 * 2048.0)
                        )
                        logits = lpool.tile([P, e], f32, tag="logits")
                        nc.vector.tensor_scalar(
                            logits,
                            pt[:, :e],
                            1.0 / 64.0,
                            None,
                            mybir.AluOpType.mult,
                        )
                        nc.vector.tensor_add(logits, logits, cross)
                        mx = spool.tile([P, TOP_K], f32, tag="mx")
                        nc.vector.max(out=mx, in_=logits)
                        idx_u = spool.tile([P, TOP_K], mybir.dt.uint32, tag="idxu")
                        nc.vector.max_index(idx_u, mx, logits)
                        nc.vector.tensor_copy(stage_idx[:, g_ts, :], idx_u)
                        # normalized top-k softmax: exp(v - v_max) / sum
                        nm = spool.tile([P, 1], f32, tag="nm")
                        nc.vector.tensor_scalar_mul(nm, mx[:, 0:1], -1.0)
                        ev = spool.tile([P, TOP_K], f32, tag="ev")
                        sm = spool.tile([P, 1], f32, tag="sm")
                        nc.scalar.activation(
                            ev,
                            mx,
                            mybir.ActivationFunctionType.Exp,
                            bias=nm,
                            scale=1.0,
                            accum_out=sm,
                        )
                        rc = spool.tile([P, 1], f32, tag="rc")
                        nc.vector.reciprocal(rc, sm)
                        nc.vector.tensor_scalar_mul(stage_wv[:, g_ts, :], ev, rc)
            nc.sync.dma_start(
                idx_out[:, :].rearrange("p (ts k) -> p ts k", k=TOP_K), stage_idx
            )
            nc.scalar.dma_start(
                w_out[:, :].rearrange("p (ts k) -> p ts k", k=TOP_K), stage_wv
            )
    nc.compile()
    return nc


def _get_nc():
    key = (T_LOC, HIDDEN, NUM_EXPERTS)
    if key not in _NC_CACHE:
        _NC_CACHE[key] = build_gate_nc(*key)
    return _NC_CACHE[key]


def _split_fp16(x, pre_scale=1.0):
    """x (f32) -> (hi, lo) fp16 with hi + lo/2^11 ~= pre_scale*x."""
    xs = x * np.float32(pre_scale) if pre_scale != 1.0 else x
    hi = xs.astype(np.float16)
    lo = ((xs - hi.astype(np.float32)) * np.float32(LO_SCALE)).astype(np.float16)
    return hi, lo


def _pack_hs(hs_part):
    """[t_loc, H] f32 -> [128, 4*56*512] fp16 hi/lo in [p][group][ko][t] order."""
    hsT = np.ascontiguousarray(hs_part.T)  # [H, t_loc]
    hi, lo = _split_fp16(hsT)
    out = []
    for x in (hi, lo):
        x4 = x.reshape(KT, P, NG, T_GRP)          # [ko][p][group][t]
        x4 = np.ascontiguousarray(x4.transpose(1, 2, 0, 3))  # [p][group][ko][t]
        out.append(x4.reshape(P, NG * KT * T_GRP))
    return out


def _prep_inputs(hs, w):
    wT = np.ascontiguousarray(w.T)  # [H, E]
    w_hi, w_lo = _split_fp16(wT, W_SCALE)
    w_cat = np.concatenate([w_hi, w_lo], axis=1)          # [H, 2E]
    w3 = w_cat.reshape(KT, P, 2 * NUM_EXPERTS)            # [ko][p][2e]
    w3 = np.ascontiguousarray(w3.transpose(1, 0, 2))      # [p][ko][2e]
    wT_packed = w3.reshape(P, KT * 2 * NUM_EXPERTS)
    in_maps = []
    for c in range(N_CORES):
        hs_hi, hs_lo = _pack_hs(hs[c * T_LOC : (c + 1) * T_LOC])
        in_maps.append({"hsT_hi": hs_hi, "hsT_lo": hs_lo, "wT_cat": wT_packed})
    return in_maps


_FN_CACHE = {}


def _make_runner(nc):
    """Compile a reusable 8-core PJRT callable (same lowering path as
    run_bass_kernel_spmd under axon, but cached so repeat kernel() calls
    skip re-tracing/compiling)."""
    import jax
    import concourse.mybir as mybir
    from concourse import bass2jax
    from jax.sharding import Mesh, NamedSharding, PartitionSpec
    from jax.experimental.shard_map import shard_map

    bass2jax.install_neuronx_cc_hook()
    partition_name = nc.partition_id_tensor.name if nc.partition_id_tensor else None
    in_names, out_names, out_avals, zero_shapes = [], [], [], []
    for alloc in nc.m.functions[0].allocations:
        if not isinstance(alloc, mybir.MemoryLocationSet):
            continue
        name = alloc.memorylocations[0].name
        if alloc.kind == "ExternalInput":
            if name != partition_name:
                in_names.append(name)
        elif alloc.kind == "ExternalOutput":
            shape = tuple(alloc.tensor_shape)
            dtype = mybir.dt.np(alloc.dtype)
            out_names.append(name)
            out_avals.append(jax.core.ShapedArray(shape, dtype))
            zero_shapes.append((shape, dtype))
    n_params = len(in_names)
    n_outs = len(out_avals)
    all_in_names = list(in_names) + list(out_names)
    if partition_name is not None:
        all_in_names.append(partition_name)

    def _body(*args):
        operands = list(args)
        if partition_name is not None:
            operands.append(bass2jax.partition_id_tensor())
        outs = bass2jax._bass_exec_p.bind(
            *operands,
            out_avals=tuple(out_avals),
            in_names=tuple(all_in_names),
            out_names=tuple(out_names),
            lowering_input_output_aliases=(),
            sim_require_finite=True,
            sim_require_nnan=True,
            nc=nc,
        )
        return tuple(outs)

    devices = jax.devices()[:N_CORES]
    mesh = Mesh(np.asarray(devices), ("core",))
    in_specs = (PartitionSpec("core"),) * (n_params + n_outs)
    out_specs = (PartitionSpec("core"),) * len(out_names)
    donate = tuple(range(n_params, n_params + n_outs))
    fn = jax.jit(
        shard_map(
            _body, mesh=mesh, in_specs=in_specs, out_specs=out_specs, check_rep=False
        ),
        donate_argnums=donate,
        keep_unused=True,
    )
    sharding = NamedSharding(mesh, PartitionSpec("core"))

    def run(in_maps):
        concat_in = [
            np.concatenate(
                [np.asarray(in_maps[c][nm]) for c in range(N_CORES)], axis=0
            )
            for nm in in_names
        ]
        zeros = [
            np.zeros((N_CORES * s[0], *s[1:]), dt) for s, dt in zero_shapes
        ]
        dev_in = [jax.device_put(x, sharding) for x in concat_in]
        out_arrs = fn(*dev_in, *zeros)
        return [
            {
                nm: np.asarray(out_arrs[i]).reshape(
                    N_CORES, *out_avals[i].shape
                )[c]
                for i, nm in enumerate(out_names)
            }
            for c in range(N_CORES)
        ]

    return run


def kernel(hidden_states, weight):
    hs = np.asarray(hidden_states, dtype=np.float32)
    w = np.asarray(weight, dtype=np.float32)
    assert hs.shape == (NUM_TOKENS, HIDDEN), hs.shape
    assert w.shape == (NUM_EXPERTS, HIDDEN), w.shape

    in_maps = _prep_inputs(hs, w)
    nc = _get_nc()
    try:
        if "run" not in _FN_CACHE:
            _FN_CACHE["run"] = _make_runner(nc)
        results = _FN_CACHE["run"](in_maps)
    except Exception:
        # fall back to the stock path if the cached-runner path breaks
        from concourse.bass_utils import run_bass_kernel_spmd

        results = run_bass_kernel_spmd(
            nc, in_maps, core_ids=list(range(N_CORES))
        ).results

    def _unstage(x):
        # [P, TS*K] staging order -> [t_loc, K] (token = ts*128 + p)
        ts_n = T_LOC // P
        return x.reshape(P, ts_n, TOP_K).transpose(1, 0, 2).reshape(T_LOC, TOP_K)

    topk_idx = np.concatenate([_unstage(r["topk_idx"]) for r in results], axis=0)
    topk_w = np.concatenate([_unstage(r["topk_w"]) for r in results], axis=0)
    row_idx = (
        np.arange(NUM_TOKENS * TOP_K, dtype=np.int32).reshape(TOP_K, NUM_TOKENS).T
    )
    return (
        topk_idx.astype(np.int32),
        topk_w.astype(np.float32),
        row_idx,
    )


# revision 15
# speedup vs baseline: 1.2117x; 1.0014x over previous
"""MoE gate (router) kernel for Trainium2 — v2: big-line DMA layout.

Computes, for hidden_states [T, H] and gate weight [E, H]:
    logits = hidden_states @ weight.T          # [T, E]
    probs  = softmax(logits, axis=-1)
    topk_weight, topk_idx = top_k(probs, 8)    # normalized over the top-8
    row_idx = arange(T*8).reshape(8, T).T

Strategy (8 NeuronCores, data parallel over tokens):
  - fp32 accuracy from fp16 hi/lo splits (host-side, same DMA bytes as f32):
    hs = hi + lo/2^11, 64*w = whi + wlo/2^11.  Three fp16 matmul terms:
        psum[:, 0:512]   += hshi . [whi | wlo]
        psum[:, 256:512] += hslo . whi          (same 2^11 scale as hi*lo)
    logits = 2^-6 * psum[:, 0:256] + 2^-17 * psum[:, 256:512]
  - v2/v3 DMA layout: the baseline DMA'd [128, KC, 128-token] tiles whose
    256-byte contiguous lines pay the <512B half-bandwidth DMA penalty
    (NTFF: DMA 98% active at ~187GB/s, 57us of PE idle).  Now hs is
    pre-arranged host-side as [p=128][group][ko][512 tokens] so each
    k-chunk DMA moves 8KB-contiguous per-partition lines at full
    bandwidth (NTFF: ~304GB/s), with hi on the SP queue and lo on the
    Activation queue across both HWDGE rings.  The gate weight loads in
    NKC chunks on the gpsimd (SWDGE) queue so the first matmul waits on
    ~1MB instead of the whole 7.3MB.
  - Tokens are processed in four 512-token groups; each group accumulates
    4 x 128-token subtiles in 4 PSUM banks across all 56 k-tiles, so two
    groups are in flight and the top-k evacuation of group g overlaps the
    matmuls of group g+1 instead of stalling the PE at group boundaries.
  - DVE max/max_index give top-8 values+indices; softmax over the full
    256 experts + top-k renorm reduces to a softmax over the top-8 logits.
"""

import numpy as np

TOP_K = 8
NUM_EXPERTS = 256
HIDDEN = 7168
NUM_TOKENS = 16384
N_CORES = 8
T_LOC = NUM_TOKENS // N_CORES

W_SCALE = 64.0       # weight pre-scale so fp16(64*w) stays normal-range
LO_SCALE = 2048.0    # 2^11: lo parts carry the next 11 mantissa bits

P = 128
KT = HIDDEN // P     # 56 k-tiles along hidden dim
NG = 4               # token groups per core
T_GRP = T_LOC // NG  # 512 tokens per group
TS_GRP = T_GRP // P  # 4 subtiles per group = 4 PSUM banks (2 groups in flight)
KC = 8               # k-tiles per DMA chunk (8KB/partition lines)
NKC = KT // KC       # 7 chunks

_NC_CACHE = {}


def build_gate_nc(t_loc=T_LOC, h=HIDDEN, e=NUM_EXPERTS, repeat=1):
    import concourse.mybir as mybir
    import concourse.tile as tile
    from concourse import bacc

    f32 = mybir.dt.float32
    fp16 = mybir.dt.float16

    nc = bacc.Bacc("TRN2", target_bir_lowering=False)
    # [p][group][ko][t]: per-partition contiguous KC*512 fp16 = 8KB chunks
    hsT_hi = nc.dram_tensor("hsT_hi", [P, NG * KT * T_GRP], fp16, kind="ExternalInput")
    hsT_lo = nc.dram_tensor("hsT_lo", [P, NG * KT * T_GRP], fp16, kind="ExternalInput")
    # [p][ko][0:256]=fp16(64*wT), [p][ko][256:512]=fp16((64*wT - hi) * 2^11)
    wT_cat = nc.dram_tensor("wT_cat", [P, KT * 2 * e], fp16, kind="ExternalInput")
    # outputs leave in on-chip staging order [p][ts][k]; host reorders to
    # [ts*128+p, k] (free) so the DMA moves one contiguous 512B line per
    # partition instead of 2048 x 32B descriptors
    idx_out = nc.dram_tensor(
        "topk_idx", [P, (t_loc // P) * TOP_K], mybir.dt.int32, kind="ExternalOutput"
    )
    w_out = nc.dram_tensor("topk_w", [P, (t_loc // P) * TOP_K], f32, kind="ExternalOutput")

    TS = t_loc // P  # 16 subtiles total

    with tile.TileContext(nc) as tc:
        with (
            tc.tile_pool(name="wpool", bufs=1) as wpool,
            tc.tile_pool(name="hpool", bufs=6) as hpool,
            tc.tile_pool(name="lpool", bufs=3) as lpool,
            tc.tile_pool(name="spool", bufs=4) as spool,
            tc.tile_pool(name="psum", bufs=8, space="PSUM") as psum_pool,
        ):
            # output staging: results accumulate here and leave as two large
            # descriptor DMAs at the end
            stage_idx = wpool.tile([P, TS, TOP_K], mybir.dt.int32, tag="sidx")
            stage_wv = wpool.tile([P, TS, TOP_K], f32, tag="swv")
            # gate weight: resident in SBUF, loaded in NKC chunks on the
            # otherwise-idle gpsimd (SWDGE) queue so the first matmul waits
            # on ~1MB, not 7.3MB.  (Routing early weight chunks via SP/Act
            # instead was tried and costs 70us/device: it delays the hs
            # stream those queues carry.)
            wt = wpool.tile([P, KT, 2 * e], fp16, tag="wt")
            wt_view = wT_cat[:, :].rearrange("p (ko e) -> p ko e", ko=KT)
            for kc in range(NKC):
                kslc = slice(kc * KC, (kc + 1) * KC)
                nc.gpsimd.dma_start(wt[:, kslc, :], wt_view[:, kslc, :])
            for rep in range(repeat):
                for grp in range(NG):
                    pts = []
                    for ts_i in range(TS_GRP):
                        pts.append(
                            psum_pool.tile(
                                [P, 2 * e], f32, tag="pt",
                                name=f"pt{rep}_{grp}_{ts_i}",
                            )
                        )
                    for kc in range(NKC):
                        base = grp * (KT * T_GRP) + kc * (KC * T_GRP)
                        hhi = hpool.tile(
                            [P, KC, T_GRP], fp16, tag="hs",
                            name=f"hshi{rep}_{grp}_{kc}",
                        )
                        nc.sync.dma_start(
                            hhi,
                            hsT_hi[:, base : base + KC * T_GRP].rearrange(
                                "p (ko t) -> p ko t", ko=KC
                            ),
                        )
                        hlo = hpool.tile(
                            [P, KC, T_GRP], fp16, tag="hs",
                            name=f"hslo{rep}_{grp}_{kc}",
                        )
                        nc.scalar.dma_start(
                            hlo,
                            hsT_lo[:, base : base + KC * T_GRP].rearrange(
                                "p (ko t) -> p ko t", ko=KC
                            ),
                        )
                        for ki in range(KC):
                            k = kc * KC + ki
                            for ts_i in range(TS_GRP):
                                pt = pts[ts_i]
                                tslc = slice(ts_i * P, (ts_i + 1) * P)
                                # psum[:, 0:2e] += hshi . [whi | wlo]
                                nc.tensor.matmul(
                                    pt,
                                    hhi[:, ki, tslc],
                                    wt[:, k, :],
                                    start=(k == 0),
                                    stop=False,
                                )
                                # psum[:, e:2e] += hslo . whi  (2^11 scale)
                                nc.tensor.matmul(
                                    pt[:, e:],
                                    hlo[:, ki, tslc],
                                    wt[:, k, :e],
                                    start=False,
                                    stop=(k == KT - 1),
                                )
                    for ts_i in range(TS_GRP):
                        pt = pts[ts_i]
                        g_ts = grp * TS_GRP + ts_i
                        # logits = 2^-6 * psum_hi + 2^-17 * psum_cross
                        cross = lpool.tile([P, e], f32, tag="cross")
                        nc.vector.tensor_scalar_mul(
                            cross, pt[:, e:], 1.0 / (64.0 * 2048.0)
                        )
                        logits = lpool.tile([P, e], f32, tag="logits")
                        nc.vector.tensor_scalar(
                            logits,
                            pt[:, :e],
                            1.0 / 64.0,
                            None,
                            mybir.AluOpType.mult,
                        )
                        nc.vector.tensor_add(logits, logits, cross)
                        mx = spool.tile([P, TOP_K], f32, tag="mx")
                        nc.vector.max(out=mx, in_=logits)
                        idx_u = spool.tile([P, TOP_K], mybir.dt.uint32, tag="idxu")
                        nc.vector.max_index(idx_u, mx, logits)
                        nc.vector.tensor_copy(stage_idx[:, g_ts, :], idx_u)
                        # normalized top-k softmax: exp(v - v_max) / sum
                        nm = spool.tile([P, 1], f32, tag="nm")
                        nc.vector.tensor_scalar_mul(nm, mx[:, 0:1], -1.0)
                        ev = spool.tile([P, TOP_K], f32, tag="ev")
                        sm = spool.tile([P, 1], f32, tag="sm")
                        nc.scalar.activation(
                            ev,
                            mx,
                            mybir.ActivationFunctionType.Exp,
                            bias=nm,
                            scale=1.0,
                            accum_out=sm,
                        )
                        rc = spool.tile([P, 1], f32, tag="rc")
                        nc.vector.reciprocal(rc, sm)
                        nc.vector.tensor_scalar_mul(stage_wv[:, g_ts, :], ev, rc)
            nc.sync.dma_start(
                idx_out[:, :].rearrange("p (ts k) -> p ts k", k=TOP_K), stage_idx
            )
            nc.scalar.dma_start(
                w_out[:, :].rearrange("p (ts k) -> p ts k", k=TOP_K), stage_wv
            )
    nc.compile()
    return nc


def _get_nc():
    key = (T_LOC, HIDDEN, NUM_EXPERTS)
    if key not in _NC_CACHE:
        _NC_CACHE[key] = build_gate_nc(*key)
    return _NC_CACHE[key]


def _split_fp16(x, pre_scale=1.0):
    """x (f32) -> (hi, lo) fp16 with hi + lo/2^11 ~= pre_scale*x."""
    xs = x * np.float32(pre_scale) if pre_scale != 1.0 else x
    hi = xs.astype(np.float16)
    lo = ((xs - hi.astype(np.float32)) * np.float32(LO_SCALE)).astype(np.float16)
    return hi, lo


def _pack_hs(hs_part):
    """[t_loc, H] f32 -> [128, 4*56*512] fp16 hi/lo in [p][group][ko][t] order."""
    hsT = np.ascontiguousarray(hs_part.T)  # [H, t_loc]
    hi, lo = _split_fp16(hsT)
    out = []
    for x in (hi, lo):
        x4 = x.reshape(KT, P, NG, T_GRP)          # [ko][p][group][t]
        x4 = np.ascontiguousarray(x4.transpose(1, 2, 0, 3))  # [p][group][ko][t]
        out.append(x4.reshape(P, NG * KT * T_GRP))
    return out


def _prep_inputs(hs, w):
    wT = np.ascontiguousarray(w.T)  # [H, E]
    w_hi, w_lo = _split_fp16(wT, W_SCALE)
    w_cat = np.concatenate([w_hi, w_lo], axis=1)          # [H, 2E]
    w3 = w_cat.reshape(KT, P, 2 * NUM_EXPERTS)            # [ko][p][2e]
    w3 = np.ascontiguousarray(w3.transpose(1, 0, 2))      # [p][ko][2e]
    wT_packed = w3.reshape(P, KT * 2 * NUM_EXPERTS)
    in_maps = []
    for c in range(N_CORES):
        hs_hi, hs_lo = _pack_hs(hs[c * T_LOC : (c + 1) * T_LOC])
        in_maps.append({"hsT_hi": hs_hi, "hsT_lo": hs_lo, "wT_cat": wT_packed})
    return in_maps


_FN_CACHE = {}


def _make_runner(nc):
    """Compile a reusable 8-core PJRT callable (same lowering path as
    run_bass_kernel_spmd under axon, but cached so repeat kernel() calls
    skip re-tracing/compiling)."""
    import jax
    import concourse.mybir as mybir
    from concourse import bass2jax
    from jax.sharding import Mesh, NamedSharding, PartitionSpec
    from jax.experimental.shard_map import shard_map

    bass2jax.install_neuronx_cc_hook()
    partition_name = nc.partition_id_tensor.name if nc.partition_id_tensor else None
    in_names, out_names, out_avals, zero_shapes = [], [], [], []
    for alloc in nc.m.functions[0].allocations:
        if not isinstance(alloc, mybir.MemoryLocationSet):
            continue
        name = alloc.memorylocations[0].name
        if alloc.kind == "ExternalInput":
            if name != partition_name:
                in_names.append(name)
        elif alloc.kind == "ExternalOutput":
            shape = tuple(alloc.tensor_shape)
            dtype = mybir.dt.np(alloc.dtype)
            out_names.append(name)
            out_avals.append(jax.core.ShapedArray(shape, dtype))
            zero_shapes.append((shape, dtype))
    n_params = len(in_names)
    n_outs = len(out_avals)
    all_in_names = list(in_names) + list(out_names)
    if partition_name is not None:
        all_in_names.append(partition_name)

    def _body(*args):
        operands = list(args)
        if partition_name is not None:
            operands.append(bass2jax.partition_id_tensor())
        outs = bass2jax._bass_exec_p.bind(
            *operands,
            out_avals=tuple(out_avals),
            in_names=tuple(all_in_names),
            out_names=tuple(out_names),
            lowering_input_output_aliases=(),
            sim_require_finite=True,
            sim_require_nnan=True,
            nc=nc,
        )
        return tuple(outs)

    devices = jax.devices()[:N_CORES]
    mesh = Mesh(np.asarray(devices), ("core",))
    in_specs = (PartitionSpec("core"),) * (n_params + n_outs)
    out_specs = (PartitionSpec("core"),) * len(out_names)
    donate = tuple(range(n_params, n_params + n_outs))
    fn = jax.jit(
        shard_map(
            _body, mesh=mesh, in_specs=in_specs, out_specs=out_specs, check_rep=False
        ),
        donate_argnums=donate,
        keep_unused=True,
    )
    sharding = NamedSharding(mesh, PartitionSpec("core"))

    def run(in_maps):
        concat_in = [
            np.concatenate(
                [np.asarray(in_maps[c][nm]) for c in range(N_CORES)], axis=0
            )
            for nm in in_names
        ]
        zeros = [
            np.zeros((N_CORES * s[0], *s[1:]), dt) for s, dt in zero_shapes
        ]
        dev_in = [jax.device_put(x, sharding) for x in concat_in]
        out_arrs = fn(*dev_in, *zeros)
        return [
            {
                nm: np.asarray(out_arrs[i]).reshape(
                    N_CORES, *out_avals[i].shape
                )[c]
                for i, nm in enumerate(out_names)
            }
            for c in range(N_CORES)
        ]

    return run


def kernel(hidden_states, weight):
    hs = np.asarray(hidden_states, dtype=np.float32)
    w = np.asarray(weight, dtype=np.float32)
    assert hs.shape == (NUM_TOKENS, HIDDEN), hs.shape
    assert w.shape == (NUM_EXPERTS, HIDDEN), w.shape

    in_maps = _prep_inputs(hs, w)
    nc = _get_nc()
    try:
        if "run" not in _FN_CACHE:
            _FN_CACHE["run"] = _make_runner(nc)
        results = _FN_CACHE["run"](in_maps)
    except Exception:
        # fall back to the stock path if the cached-runner path breaks
        from concourse.bass_utils import run_bass_kernel_spmd

        results = run_bass_kernel_spmd(
            nc, in_maps, core_ids=list(range(N_CORES))
        ).results

    def _unstage(x):
        # [P, TS*K] staging order -> [t_loc, K] (token = ts*128 + p)
        ts_n = T_LOC // P
        return x.reshape(P, ts_n, TOP_K).transpose(1, 0, 2).reshape(T_LOC, TOP_K)

    topk_idx = np.concatenate([_unstage(r["topk_idx"]) for r in results], axis=0)
    topk_w = np.concatenate([_unstage(r["topk_w"]) for r in results], axis=0)
    row_idx = (
        np.arange(NUM_TOKENS * TOP_K, dtype=np.int32).reshape(TOP_K, NUM_TOKENS).T
    )
    return (
        topk_idx.astype(np.int32),
        topk_w.astype(np.float32),
        row_idx,
    )


# revision 16
# speedup vs baseline: 1.2149x; 1.0027x over previous
"""MoE gate (router) kernel for Trainium2 — v2: big-line DMA layout.

Computes, for hidden_states [T, H] and gate weight [E, H]:
    logits = hidden_states @ weight.T          # [T, E]
    probs  = softmax(logits, axis=-1)
    topk_weight, topk_idx = top_k(probs, 8)    # normalized over the top-8
    row_idx = arange(T*8).reshape(8, T).T

Strategy (8 NeuronCores, data parallel over tokens):
  - fp32 accuracy from fp16 hi/lo splits (host-side, same DMA bytes as f32):
    hs = hi + lo/2^11, 64*w = whi + wlo/2^11.  Three fp16 matmul terms:
        psum[:, 0:512]   += hshi . [whi | wlo]
        psum[:, 256:512] += hslo . whi          (same 2^11 scale as hi*lo)
    logits = 2^-6 * psum[:, 0:256] + 2^-17 * psum[:, 256:512]
  - v2/v3 DMA layout: the baseline DMA'd [128, KC, 128-token] tiles whose
    256-byte contiguous lines pay the <512B half-bandwidth DMA penalty
    (NTFF: DMA 98% active at ~187GB/s, 57us of PE idle).  Now hs is
    pre-arranged host-side as [p=128][group][ko][512 tokens] so each
    k-chunk DMA moves 8KB-contiguous per-partition lines at full
    bandwidth (NTFF: ~304GB/s), with hi on the SP queue and lo on the
    Activation queue across both HWDGE rings.  The gate weight loads in
    NKC chunks on the gpsimd (SWDGE) queue so the first matmul waits on
    ~1MB instead of the whole 7.3MB.
  - Tokens are processed in four 512-token groups; each group accumulates
    4 x 128-token subtiles in 4 PSUM banks across all 56 k-tiles, so two
    groups are in flight and the top-k evacuation of group g overlaps the
    matmuls of group g+1 instead of stalling the PE at group boundaries.
  - DVE max/max_index give top-8 values+indices; softmax over the full
    256 experts + top-k renorm reduces to a softmax over the top-8 logits.
"""

import numpy as np

TOP_K = 8
NUM_EXPERTS = 256
HIDDEN = 7168
NUM_TOKENS = 16384
N_CORES = 8
T_LOC = NUM_TOKENS // N_CORES

W_SCALE = 64.0       # weight pre-scale so fp16(64*w) stays normal-range
LO_SCALE = 2048.0    # 2^11: lo parts carry the next 11 mantissa bits

P = 128
KT = HIDDEN // P     # 56 k-tiles along hidden dim
NG = 4               # token groups per core
T_GRP = T_LOC // NG  # 512 tokens per group
TS_GRP = T_GRP // P  # 4 subtiles per group = 4 PSUM banks (2 groups in flight)
KC = 8               # k-tiles per DMA chunk (8KB/partition lines)
NKC = KT // KC       # 7 chunks

_NC_CACHE = {}


def build_gate_nc(t_loc=T_LOC, h=HIDDEN, e=NUM_EXPERTS, repeat=1):
    import concourse.mybir as mybir
    import concourse.tile as tile
    from concourse import bacc

    f32 = mybir.dt.float32
    fp16 = mybir.dt.float16

    nc = bacc.Bacc("TRN2", target_bir_lowering=False)
    # [p][group][ko][t]: per-partition contiguous KC*512 fp16 = 8KB chunks
    hsT_hi = nc.dram_tensor("hsT_hi", [P, NG * KT * T_GRP], fp16, kind="ExternalInput")
    hsT_lo = nc.dram_tensor("hsT_lo", [P, NG * KT * T_GRP], fp16, kind="ExternalInput")
    # [p][ko][0:256]=fp16(64*wT), [p][ko][256:512]=fp16((64*wT - hi) * 2^11)
    wT_cat = nc.dram_tensor("wT_cat", [P, KT * 2 * e], fp16, kind="ExternalInput")
    # outputs leave in on-chip staging order [p][ts][k]; host reorders to
    # [ts*128+p, k] (free) so the DMA moves one contiguous 512B line per
    # partition instead of 2048 x 32B descriptors
    idx_out = nc.dram_tensor(
        "topk_idx", [P, (t_loc // P) * TOP_K], mybir.dt.int32, kind="ExternalOutput"
    )
    w_out = nc.dram_tensor("topk_w", [P, (t_loc // P) * TOP_K], f32, kind="ExternalOutput")

    TS = t_loc // P  # 16 subtiles total

    with tile.TileContext(nc) as tc:
        with (
            tc.tile_pool(name="wpool", bufs=1) as wpool,
            tc.tile_pool(name="hpool", bufs=6) as hpool,
            tc.tile_pool(name="lpool", bufs=3) as lpool,
            tc.tile_pool(name="spool", bufs=4) as spool,
            tc.tile_pool(name="psum", bufs=8, space="PSUM") as psum_pool,
        ):
            # output staging: results accumulate here and leave as two large
            # descriptor DMAs at the end
            stage_idx = wpool.tile([P, TS, TOP_K], mybir.dt.int32, tag="sidx")
            stage_wv = wpool.tile([P, TS, TOP_K], f32, tag="swv")
            # gate weight: resident in SBUF, loaded in NKC chunks on the
            # otherwise-idle gpsimd (SWDGE) queue so the first matmul waits
            # on ~1MB, not 7.3MB.  (Routing early weight chunks via SP/Act
            # instead was tried and costs 70us/device: it delays the hs
            # stream those queues carry.)
            wt = wpool.tile([P, KT, 2 * e], fp16, tag="wt")
            wt_view = wT_cat[:, :].rearrange("p (ko e) -> p ko e", ko=KT)
            for kc in range(NKC):
                kslc = slice(kc * KC, (kc + 1) * KC)
                nc.gpsimd.dma_start(wt[:, kslc, :], wt_view[:, kslc, :])
            for rep in range(repeat):
                for grp in range(NG):
                    pts = []
                    for ts_i in range(TS_GRP):
                        pts.append(
                            psum_pool.tile(
                                [P, 2 * e], f32, tag="pt",
                                name=f"pt{rep}_{grp}_{ts_i}",
                            )
                        )
                    for kc in range(NKC):
                        base = grp * (KT * T_GRP) + kc * (KC * T_GRP)
                        hhi = hpool.tile(
                            [P, KC, T_GRP], fp16, tag="hs",
                            name=f"hshi{rep}_{grp}_{kc}",
                        )
                        nc.sync.dma_start(
                            hhi,
                            hsT_hi[:, base : base + KC * T_GRP].rearrange(
                                "p (ko t) -> p ko t", ko=KC
                            ),
                        )
                        hlo = hpool.tile(
                            [P, KC, T_GRP], fp16, tag="hs",
                            name=f"hslo{rep}_{grp}_{kc}",
                        )
                        nc.scalar.dma_start(
                            hlo,
                            hsT_lo[:, base : base + KC * T_GRP].rearrange(
                                "p (ko t) -> p ko t", ko=KC
                            ),
                        )
                        for ki in range(KC):
                            k = kc * KC + ki
                            for ts_i in range(TS_GRP):
                                pt = pts[ts_i]
                                tslc = slice(ts_i * P, (ts_i + 1) * P)
                                # psum[:, 0:2e] += hshi . [whi | wlo]
                                nc.tensor.matmul(
                                    pt,
                                    hhi[:, ki, tslc],
                                    wt[:, k, :],
                                    start=(k == 0),
                                    stop=False,
                                )
                                # psum[:, e:2e] += hslo . whi  (2^11 scale)
                                nc.tensor.matmul(
                                    pt[:, e:],
                                    hlo[:, ki, tslc],
                                    wt[:, k, :e],
                                    start=False,
                                    stop=(k == KT - 1),
                                )
                    for ts_i in range(TS_GRP):
                        pt = pts[ts_i]
                        g_ts = grp * TS_GRP + ts_i
                        # work on 64x-scaled logits: top-k selection is
                        # monotonic under the scale, and the 1/64 folds into
                        # the Exp activation below — saves one DVE pass
                        # logits64 = psum_hi + 2^-11 * psum_cross
                        cross = lpool.tile([P, e], f32, tag="cross")
                        nc.vector.tensor_scalar_mul(cross, pt[:, e:], 1.0 / 2048.0)
                        logits = lpool.tile([P, e], f32, tag="logits")
                        nc.vector.tensor_add(logits, pt[:, :e], cross)
                        mx = spool.tile([P, TOP_K], f32, tag="mx")
                        nc.vector.max(out=mx, in_=logits)
                        idx_u = spool.tile([P, TOP_K], mybir.dt.uint32, tag="idxu")
                        nc.vector.max_index(idx_u, mx, logits)
                        nc.vector.tensor_copy(stage_idx[:, g_ts, :], idx_u)
                        # normalized top-k softmax: exp((v - v_max)/64) / sum
                        nm = spool.tile([P, 1], f32, tag="nm")
                        nc.vector.tensor_scalar_mul(nm, mx[:, 0:1], -1.0 / 64.0)
                        ev = spool.tile([P, TOP_K], f32, tag="ev")
                        sm = spool.tile([P, 1], f32, tag="sm")
                        nc.scalar.activation(
                            ev,
                            mx,
                            mybir.ActivationFunctionType.Exp,
                            bias=nm,
                            scale=1.0 / 64.0,
                            accum_out=sm,
                        )
                        rc = spool.tile([P, 1], f32, tag="rc")
                        nc.vector.reciprocal(rc, sm)
                        nc.vector.tensor_scalar_mul(stage_wv[:, g_ts, :], ev, rc)
            nc.sync.dma_start(
                idx_out[:, :].rearrange("p (ts k) -> p ts k", k=TOP_K), stage_idx
            )
            nc.scalar.dma_start(
                w_out[:, :].rearrange("p (ts k) -> p ts k", k=TOP_K), stage_wv
            )
    nc.compile()
    return nc


def _get_nc():
    key = (T_LOC, HIDDEN, NUM_EXPERTS)
    if key not in _NC_CACHE:
        _NC_CACHE[key] = build_gate_nc(*key)
    return _NC_CACHE[key]


def _split_fp16(x, pre_scale=1.0):
    """x (f32) -> (hi, lo) fp16 with hi + lo/2^11 ~= pre_scale*x."""
    xs = x * np.float32(pre_scale) if pre_scale != 1.0 else x
    hi = xs.astype(np.float16)
    lo = ((xs - hi.astype(np.float32)) * np.float32(LO_SCALE)).astype(np.float16)
    return hi, lo


def _pack_hs(hs_part):
    """[t_loc, H] f32 -> [128, 4*56*512] fp16 hi/lo in [p][group][ko][t] order."""
    hsT = np.ascontiguousarray(hs_part.T)  # [H, t_loc]
    hi, lo = _split_fp16(hsT)
    out = []
    for x in (hi, lo):
        x4 = x.reshape(KT, P, NG, T_GRP)          # [ko][p][group][t]
        x4 = np.ascontiguousarray(x4.transpose(1, 2, 0, 3))  # [p][group][ko][t]
        out.append(x4.reshape(P, NG * KT * T_GRP))
    return out


def _prep_inputs(hs, w):
    wT = np.ascontiguousarray(w.T)  # [H, E]
    w_hi, w_lo = _split_fp16(wT, W_SCALE)
    w_cat = np.concatenate([w_hi, w_lo], axis=1)          # [H, 2E]
    w3 = w_cat.reshape(KT, P, 2 * NUM_EXPERTS)            # [ko][p][2e]
    w3 = np.ascontiguousarray(w3.transpose(1, 0, 2))      # [p][ko][2e]
    wT_packed = w3.reshape(P, KT * 2 * NUM_EXPERTS)
    in_maps = []
    for c in range(N_CORES):
        hs_hi, hs_lo = _pack_hs(hs[c * T_LOC : (c + 1) * T_LOC])
        in_maps.append({"hsT_hi": hs_hi, "hsT_lo": hs_lo, "wT_cat": wT_packed})
    return in_maps


_FN_CACHE = {}


def _make_runner(nc):
    """Compile a reusable 8-core PJRT callable (same lowering path as
    run_bass_kernel_spmd under axon, but cached so repeat kernel() calls
    skip re-tracing/compiling)."""
    import jax
    import concourse.mybir as mybir
    from concourse import bass2jax
    from jax.sharding import Mesh, NamedSharding, PartitionSpec
    from jax.experimental.shard_map import shard_map

    bass2jax.install_neuronx_cc_hook()
    partition_name = nc.partition_id_tensor.name if nc.partition_id_tensor else None
    in_names, out_names, out_avals, zero_shapes = [], [], [], []
    for alloc in nc.m.functions[0].allocations:
        if not isinstance(alloc, mybir.MemoryLocationSet):
            continue
        name = alloc.memorylocations[0].name
        if alloc.kind == "ExternalInput":
            if name != partition_name:
                in_names.append(name)
        elif alloc.kind == "ExternalOutput":
            shape = tuple(alloc.tensor_shape)
            dtype = mybir.dt.np(alloc.dtype)
            out_names.append(name)
            out_avals.append(jax.core.ShapedArray(shape, dtype))
            zero_shapes.append((shape, dtype))
    n_params = len(in_names)
    n_outs = len(out_avals)
    all_in_names = list(in_names) + list(out_names)
    if partition_name is not None:
        all_in_names.append(partition_name)

    def _body(*args):
        operands = list(args)
        if partition_name is not None:
            operands.append(bass2jax.partition_id_tensor())
        outs = bass2jax._bass_exec_p.bind(
            *operands,
            out_avals=tuple(out_avals),
            in_names=tuple(all_in_names),
            out_names=tuple(out_names),
            lowering_input_output_aliases=(),
            sim_require_finite=True,
            sim_require_nnan=True,
            nc=nc,
        )
        return tuple(outs)

    devices = jax.devices()[:N_CORES]
    mesh = Mesh(np.asarray(devices), ("core",))
    in_specs = (PartitionSpec("core"),) * (n_params + n_outs)
    out_specs = (PartitionSpec("core"),) * len(out_names)
    donate = tuple(range(n_params, n_params + n_outs))
    fn = jax.jit(
        shard_map(
            _body, mesh=mesh, in_specs=in_specs, out_specs=out_specs, check_rep=False
        ),
        donate_argnums=donate,
        keep_unused=True,
    )
    sharding = NamedSharding(mesh, PartitionSpec("core"))

    def run(in_maps):
        concat_in = [
            np.concatenate(
                [np.asarray(in_maps[c][nm]) for c in range(N_CORES)], axis=0
            )
            for nm in in_names
        ]
        zeros = [
            np.zeros((N_CORES * s[0], *s[1:]), dt) for s, dt in zero_shapes
        ]
        dev_in = [jax.device_put(x, sharding) for x in concat_in]
        out_arrs = fn(*dev_in, *zeros)
        return [
            {
                nm: np.asarray(out_arrs[i]).reshape(
                    N_CORES, *out_avals[i].shape
                )[c]
                for i, nm in enumerate(out_names)
            }
            for c in range(N_CORES)
        ]

    return run


def kernel(hidden_states, weight):
    hs = np.asarray(hidden_states, dtype=np.float32)
    w = np.asarray(weight, dtype=np.float32)
    assert hs.shape == (NUM_TOKENS, HIDDEN), hs.shape
    assert w.shape == (NUM_EXPERTS, HIDDEN), w.shape

    in_maps = _prep_inputs(hs, w)
    nc = _get_nc()
    try:
        if "run" not in _FN_CACHE:
            _FN_CACHE["run"] = _make_runner(nc)
        results = _FN_CACHE["run"](in_maps)
    except Exception:
        # fall back to the stock path if the cached-runner path breaks
        from concourse.bass_utils import run_bass_kernel_spmd

        results = run_bass_kernel_spmd(
            nc, in_maps, core_ids=list(range(N_CORES))
        ).results

    def _unstage(x):
        # [P, TS*K] staging order -> [t_loc, K] (token = ts*128 + p)
        ts_n = T_LOC // P
        return x.reshape(P, ts_n, TOP_K).transpose(1, 0, 2).reshape(T_LOC, TOP_K)

    topk_idx = np.concatenate([_unstage(r["topk_idx"]) for r in results], axis=0)
    topk_w = np.concatenate([_unstage(r["topk_w"]) for r in results], axis=0)
    row_idx = (
        np.arange(NUM_TOKENS * TOP_K, dtype=np.int32).reshape(TOP_K, NUM_TOKENS).T
    )
    return (
        topk_idx.astype(np.int32),
        topk_w.astype(np.float32),
        row_idx,
    )


# revision 20
# speedup vs baseline: 1.2208x; 1.0049x over previous
"""MoE gate (router) kernel for Trainium2 — v2: big-line DMA layout.

Computes, for hidden_states [T, H] and gate weight [E, H]:
    logits = hidden_states @ weight.T          # [T, E]
    probs  = softmax(logits, axis=-1)
    topk_weight, topk_idx = top_k(probs, 8)    # normalized over the top-8
    row_idx = arange(T*8).reshape(8, T).T

Strategy (8 NeuronCores, data parallel over tokens):
  - fp32 accuracy from fp16 hi/lo splits (host-side, same DMA bytes as f32):
    hs = hi + lo/2^11, 64*w = whi + wlo/2^11.  Three fp16 matmul terms:
        psum[:, 0:512]   += hshi . [whi | wlo]
        psum[:, 256:512] += hslo . whi          (same 2^11 scale as hi*lo)
    logits = 2^-6 * psum[:, 0:256] + 2^-17 * psum[:, 256:512]
  - v2/v3 DMA layout: the baseline DMA'd [128, KC, 128-token] tiles whose
    256-byte contiguous lines pay the <512B half-bandwidth DMA penalty
    (NTFF: DMA 98% active at ~187GB/s, 57us of PE idle).  Now hs is
    pre-arranged host-side as [p=128][group][ko][512 tokens] so each
    k-chunk DMA moves 8KB-contiguous per-partition lines at full
    bandwidth (NTFF: ~304GB/s), with hi on the SP queue and lo on the
    Activation queue across both HWDGE rings.  The gate weight loads in
    NKC chunks on the gpsimd (SWDGE) queue so the first matmul waits on
    ~1MB instead of the whole 7.3MB.
  - Tokens are processed in four 512-token groups; each group accumulates
    4 x 128-token subtiles in 4 PSUM banks across all 56 k-tiles, so two
    groups are in flight and the top-k evacuation of group g overlaps the
    matmuls of group g+1 instead of stalling the PE at group boundaries.
  - DVE max/max_index give top-8 values+indices; softmax over the full
    256 experts + top-k renorm reduces to a softmax over the top-8 logits.
"""

import numpy as np

TOP_K = 8
NUM_EXPERTS = 256
HIDDEN = 7168
NUM_TOKENS = 16384
N_CORES = 8
T_LOC = NUM_TOKENS // N_CORES

W_SCALE = 64.0       # weight pre-scale so fp16(64*w) stays normal-range
LO_SCALE = 2048.0    # 2^11: lo parts carry the next 11 mantissa bits

P = 128
KT = HIDDEN // P     # 56 k-tiles along hidden dim
NG = 4               # token groups per core
T_GRP = T_LOC // NG  # 512 tokens per group
TS_GRP = T_GRP // P  # 4 subtiles per group = 4 PSUM banks (2 groups in flight)
KC = 8               # k-tiles per DMA chunk (8KB/partition lines)
NKC = KT // KC       # 7 chunks

_NC_CACHE = {}


def build_gate_nc(t_loc=T_LOC, h=HIDDEN, e=NUM_EXPERTS, repeat=1):
    import concourse.mybir as mybir
    import concourse.tile as tile
    from concourse import bacc

    f32 = mybir.dt.float32
    fp16 = mybir.dt.float16

    nc = bacc.Bacc("TRN2", target_bir_lowering=False)
    # [p][group][ko][t]: per-partition contiguous KC*512 fp16 = 8KB chunks
    hsT_hi = nc.dram_tensor("hsT_hi", [P, NG * KT * T_GRP], fp16, kind="ExternalInput")
    hsT_lo = nc.dram_tensor("hsT_lo", [P, NG * KT * T_GRP], fp16, kind="ExternalInput")
    # [p][ko][0:256]=fp16(64*wT), [p][ko][256:512]=fp16((64*wT - hi) * 2^11)
    wT_cat = nc.dram_tensor("wT_cat", [P, KT * 2 * e], fp16, kind="ExternalInput")
    # outputs leave in on-chip staging order [p][ts][k]; host reorders to
    # [ts*128+p, k] (free) so the DMA moves one contiguous 512B line per
    # partition instead of 2048 x 32B descriptors
    idx_out = nc.dram_tensor(
        "topk_idx", [P, (t_loc // P) * TOP_K], mybir.dt.int32, kind="ExternalOutput"
    )
    w_out = nc.dram_tensor("topk_w", [P, (t_loc // P) * TOP_K], f32, kind="ExternalOutput")

    TS = t_loc // P  # 16 subtiles total

    with tile.TileContext(nc) as tc:
        with (
            tc.tile_pool(name="wpool", bufs=1) as wpool,
            tc.tile_pool(name="hpool", bufs=6) as hpool,
            tc.tile_pool(name="lpool", bufs=3) as lpool,
            tc.tile_pool(name="spool", bufs=4) as spool,
            tc.tile_pool(name="psum", bufs=8, space="PSUM") as psum_pool,
        ):
            # output staging: results accumulate here and leave as two large
            # descriptor DMAs at the end
            stage_idx = wpool.tile([P, TS, TOP_K], mybir.dt.int32, tag="sidx")
            stage_wv = wpool.tile([P, TS, TOP_K], f32, tag="swv")
            # gate weight: resident in SBUF, loaded in NKC chunks on the
            # otherwise-idle gpsimd (SWDGE) queue so the first matmul waits
            # on ~1MB, not 7.3MB.  (Routing early weight chunks via SP/Act
            # instead was tried and costs 70us/device: it delays the hs
            # stream those queues carry.)
            wt = wpool.tile([P, KT, 2 * e], fp16, tag="wt")
            wt_view = wT_cat[:, :].rearrange("p (ko e) -> p ko e", ko=KT)
            for kc in range(NKC):
                kslc = slice(kc * KC, (kc + 1) * KC)
                nc.gpsimd.dma_start(wt[:, kslc, :], wt_view[:, kslc, :])
            for rep in range(repeat):
                for grp in range(NG):
                    pts = []
                    for ts_i in range(TS_GRP):
                        pts.append(
                            psum_pool.tile(
                                [P, 2 * e], f32, tag="pt",
                                name=f"pt{rep}_{grp}_{ts_i}",
                            )
                        )
                    for kc in range(NKC):
                        base = grp * (KT * T_GRP) + kc * (KC * T_GRP)
                        hhi = hpool.tile(
                            [P, KC, T_GRP], fp16, tag="hs",
                            name=f"hshi{rep}_{grp}_{kc}",
                        )
                        nc.sync.dma_start(
                            hhi,
                            hsT_hi[:, base : base + KC * T_GRP].rearrange(
                                "p (ko t) -> p ko t", ko=KC
                            ),
                        )
                        hlo = hpool.tile(
                            [P, KC, T_GRP], fp16, tag="hs",
                            name=f"hslo{rep}_{grp}_{kc}",
                        )
                        nc.scalar.dma_start(
                            hlo,
                            hsT_lo[:, base : base + KC * T_GRP].rearrange(
                                "p (ko t) -> p ko t", ko=KC
                            ),
                        )
                        # last chunk runs ts-outer so bank ts_i finishes its
                        # full K-reduction ~2.7us before bank ts_i+1: the
                        # group's first three evacs overlap the remaining
                        # matmuls and only the final evac is tail-exposed
                        if kc == NKC - 1:
                            order = [
                                (ki, ts_i)
                                for ts_i in range(TS_GRP)
                                for ki in range(KC)
                            ]
                        else:
                            order = [
                                (ki, ts_i)
                                for ki in range(KC)
                                for ts_i in range(TS_GRP)
                            ]
                        for ki, ts_i in order:
                            k = kc * KC + ki
                            pt = pts[ts_i]
                            tslc = slice(ts_i * P, (ts_i + 1) * P)
                            # psum[:, 0:2e] += hshi . [whi | wlo]
                            nc.tensor.matmul(
                                pt,
                                hhi[:, ki, tslc],
                                wt[:, k, :],
                                start=(k == 0),
                                stop=False,
                            )
                            # psum[:, e:2e] += hslo . whi  (2^11 scale)
                            nc.tensor.matmul(
                                pt[:, e:],
                                hlo[:, ki, tslc],
                                wt[:, k, :e],
                                start=False,
                                stop=(k == KT - 1),
                            )
                    for ts_i in range(TS_GRP):
                        pt = pts[ts_i]
                        g_ts = grp * TS_GRP + ts_i
                        # work on 64x-scaled logits: top-k selection is
                        # monotonic under the scale, and the 1/64 folds into
                        # the Exp activation below — saves one DVE pass
                        # logits64 = psum_hi + 2^-11 * psum_cross
                        cross = lpool.tile([P, e], f32, tag="cross")
                        nc.vector.tensor_scalar_mul(cross, pt[:, e:], 1.0 / 2048.0)
                        logits = lpool.tile([P, e], f32, tag="logits")
                        nc.vector.tensor_add(logits, pt[:, :e], cross)
                        mx = spool.tile([P, TOP_K], f32, tag="mx")
                        nc.vector.max(out=mx, in_=logits)
                        idx_u = spool.tile([P, TOP_K], mybir.dt.uint32, tag="idxu")
                        nc.vector.max_index(idx_u, mx, logits)
                        nc.vector.tensor_copy(stage_idx[:, g_ts, :], idx_u)
                        # normalized top-k softmax: exp((v - v_max)/64) / sum
                        nm = spool.tile([P, 1], f32, tag="nm")
                        nc.vector.tensor_scalar_mul(nm, mx[:, 0:1], -1.0 / 64.0)
                        ev = spool.tile([P, TOP_K], f32, tag="ev")
                        sm = spool.tile([P, 1], f32, tag="sm")
                        nc.scalar.activation(
                            ev,
                            mx,
                            mybir.ActivationFunctionType.Exp,
                            bias=nm,
                            scale=1.0 / 64.0,
                            accum_out=sm,
                        )
                        rc = spool.tile([P, 1], f32, tag="rc")
                        nc.vector.reciprocal(rc, sm)
                        nc.vector.tensor_scalar_mul(stage_wv[:, g_ts, :], ev, rc)
            nc.sync.dma_start(
                idx_out[:, :].rearrange("p (ts k) -> p ts k", k=TOP_K), stage_idx
            )
            nc.scalar.dma_start(
                w_out[:, :].rearrange("p (ts k) -> p ts k", k=TOP_K), stage_wv
            )
    nc.compile()
    return nc


def _get_nc():
    key = (T_LOC, HIDDEN, NUM_EXPERTS)
    if key not in _NC_CACHE:
        _NC_CACHE[key] = build_gate_nc(*key)
    return _NC_CACHE[key]


def _split_fp16(x, pre_scale=1.0):
    """x (f32) -> (hi, lo) fp16 with hi + lo/2^11 ~= pre_scale*x."""
    xs = x * np.float32(pre_scale) if pre_scale != 1.0 else x
    hi = xs.astype(np.float16)
    lo = ((xs - hi.astype(np.float32)) * np.float32(LO_SCALE)).astype(np.float16)
    return hi, lo


def _pack_hs(hs_part):
    """[t_loc, H] f32 -> [128, 4*56*512] fp16 hi/lo in [p][group][ko][t] order."""
    hsT = np.ascontiguousarray(hs_part.T)  # [H, t_loc]
    hi, lo = _split_fp16(hsT)
    out = []
    for x in (hi, lo):
        x4 = x.reshape(KT, P, NG, T_GRP)          # [ko][p][group][t]
        x4 = np.ascontiguousarray(x4.transpose(1, 2, 0, 3))  # [p][group][ko][t]
        out.append(x4.reshape(P, NG * KT * T_GRP))
    return out


def _prep_inputs(hs, w):
    wT = np.ascontiguousarray(w.T)  # [H, E]
    w_hi, w_lo = _split_fp16(wT, W_SCALE)
    w_cat = np.concatenate([w_hi, w_lo], axis=1)          # [H, 2E]
    w3 = w_cat.reshape(KT, P, 2 * NUM_EXPERTS)            # [ko][p][2e]
    w3 = np.ascontiguousarray(w3.transpose(1, 0, 2))      # [p][ko][2e]
    wT_packed = w3.reshape(P, KT * 2 * NUM_EXPERTS)
    in_maps = []
    for c in range(N_CORES):
        hs_hi, hs_lo = _pack_hs(hs[c * T_LOC : (c + 1) * T_LOC])
        in_maps.append({"hsT_hi": hs_hi, "hsT_lo": hs_lo, "wT_cat": wT_packed})
    return in_maps


_FN_CACHE = {}


def _make_runner(nc):
    """Compile a reusable 8-core PJRT callable (same lowering path as
    run_bass_kernel_spmd under axon, but cached so repeat kernel() calls
    skip re-tracing/compiling)."""
    import jax
    import concourse.mybir as mybir
    from concourse import bass2jax
    from jax.sharding import Mesh, NamedSharding, PartitionSpec
    from jax.experimental.shard_map import shard_map

    bass2jax.install_neuronx_cc_hook()
    partition_name = nc.partition_id_tensor.name if nc.partition_id_tensor else None
    in_names, out_names, out_avals, zero_shapes = [], [], [], []
    for alloc in nc.m.functions[0].allocations:
        if not isinstance(alloc, mybir.MemoryLocationSet):
            continue
        name = alloc.memorylocations[0].name
        if alloc.kind == "ExternalInput":
            if name != partition_name:
                in_names.append(name)
        elif alloc.kind == "ExternalOutput":
            shape = tuple(alloc.tensor_shape)
            dtype = mybir.dt.np(alloc.dtype)
            out_names.append(name)
            out_avals.append(jax.core.ShapedArray(shape, dtype))
            zero_shapes.append((shape, dtype))
    n_params = len(in_names)
    n_outs = len(out_avals)
    all_in_names = list(in_names) + list(out_names)
    if partition_name is not None:
        all_in_names.append(partition_name)

    def _body(*args):
        operands = list(args)
        if partition_name is not None:
            operands.append(bass2jax.partition_id_tensor())
        outs = bass2jax._bass_exec_p.bind(
            *operands,
            out_avals=tuple(out_avals),
            in_names=tuple(all_in_names),
            out_names=tuple(out_names),
            lowering_input_output_aliases=(),
            sim_require_finite=True,
            sim_require_nnan=True,
            nc=nc,
        )
        return tuple(outs)

    devices = jax.devices()[:N_CORES]
    mesh = Mesh(np.asarray(devices), ("core",))
    in_specs = (PartitionSpec("core"),) * (n_params + n_outs)
    out_specs = (PartitionSpec("core"),) * len(out_names)
    donate = tuple(range(n_params, n_params + n_outs))
    fn = jax.jit(
        shard_map(
            _body, mesh=mesh, in_specs=in_specs, out_specs=out_specs, check_rep=False
        ),
        donate_argnums=donate,
        keep_unused=True,
    )
    sharding = NamedSharding(mesh, PartitionSpec("core"))

    def run(in_maps):
        concat_in = [
            np.concatenate(
                [np.asarray(in_maps[c][nm]) for c in range(N_CORES)], axis=0
            )
            for nm in in_names
        ]
        zeros = [
            np.zeros((N_CORES * s[0], *s[1:]), dt) for s, dt in zero_shapes
        ]
        dev_in = [jax.device_put(x, sharding) for x in concat_in]
        out_arrs = fn(*dev_in, *zeros)
        return [
            {
                nm: np.asarray(out_arrs[i]).reshape(
                    N_CORES, *out_avals[i].shape
                )[c]
                for i, nm in enumerate(out_names)
            }
            for c in range(N_CORES)
        ]

    return run


def kernel(hidden_states, weight):
    hs = np.asarray(hidden_states, dtype=np.float32)
    w = np.asarray(weight, dtype=np.float32)
    assert hs.shape == (NUM_TOKENS, HIDDEN), hs.shape
    assert w.shape == (NUM_EXPERTS, HIDDEN), w.shape

    in_maps = _prep_inputs(hs, w)
    nc = _get_nc()
    try:
        if "run" not in _FN_CACHE:
            _FN_CACHE["run"] = _make_runner(nc)
        results = _FN_CACHE["run"](in_maps)
    except Exception:
        # fall back to the stock path if the cached-runner path breaks
        from concourse.bass_utils import run_bass_kernel_spmd

        results = run_bass_kernel_spmd(
            nc, in_maps, core_ids=list(range(N_CORES))
        ).results

    def _unstage(x):
        # [P, TS*K] staging order -> [t_loc, K] (token = ts*128 + p)
        ts_n = T_LOC // P
        return x.reshape(P, ts_n, TOP_K).transpose(1, 0, 2).reshape(T_LOC, TOP_K)

    topk_idx = np.concatenate([_unstage(r["topk_idx"]) for r in results], axis=0)
    topk_w = np.concatenate([_unstage(r["topk_w"]) for r in results], axis=0)
    row_idx = (
        np.arange(NUM_TOKENS * TOP_K, dtype=np.int32).reshape(TOP_K, NUM_TOKENS).T
    )
    return (
        topk_idx.astype(np.int32),
        topk_w.astype(np.float32),
        row_idx,
    )


# revision 21
# speedup vs baseline: 1.2257x; 1.0040x over previous
"""MoE gate (router) kernel for Trainium2 — v2: big-line DMA layout.

Computes, for hidden_states [T, H] and gate weight [E, H]:
    logits = hidden_states @ weight.T          # [T, E]
    probs  = softmax(logits, axis=-1)
    topk_weight, topk_idx = top_k(probs, 8)    # normalized over the top-8
    row_idx = arange(T*8).reshape(8, T).T

Strategy (8 NeuronCores, data parallel over tokens):
  - fp32 accuracy from fp16 hi/lo splits (host-side, same DMA bytes as f32):
    hs = hi + lo/2^11, 64*w = whi + wlo/2^11.  Three fp16 matmul terms:
        psum[:, 0:512]   += hshi . [whi | wlo]
        psum[:, 256:512] += hslo . whi          (same 2^11 scale as hi*lo)
    logits = 2^-6 * psum[:, 0:256] + 2^-17 * psum[:, 256:512]
  - v2/v3 DMA layout: the baseline DMA'd [128, KC, 128-token] tiles whose
    256-byte contiguous lines pay the <512B half-bandwidth DMA penalty
    (NTFF: DMA 98% active at ~187GB/s, 57us of PE idle).  Now hs is
    pre-arranged host-side as [p=128][group][ko][512 tokens] so each
    k-chunk DMA moves 8KB-contiguous per-partition lines at full
    bandwidth (NTFF: ~304GB/s), with hi on the SP queue and lo on the
    Activation queue across both HWDGE rings.  The gate weight loads in
    NKC chunks on the gpsimd (SWDGE) queue so the first matmul waits on
    ~1MB instead of the whole 7.3MB.
  - Tokens are processed in four 512-token groups; each group accumulates
    4 x 128-token subtiles in 4 PSUM banks across all 56 k-tiles, so two
    groups are in flight and the top-k evacuation of group g overlaps the
    matmuls of group g+1 instead of stalling the PE at group boundaries.
  - DVE max/max_index give top-8 values+indices; softmax over the full
    256 experts + top-k renorm reduces to a softmax over the top-8 logits.
"""

import numpy as np

TOP_K = 8
NUM_EXPERTS = 256
HIDDEN = 7168
NUM_TOKENS = 16384
N_CORES = 8
T_LOC = NUM_TOKENS // N_CORES

W_SCALE = 64.0       # weight pre-scale so fp16(64*w) stays normal-range
LO_SCALE = 2048.0    # 2^11: lo parts carry the next 11 mantissa bits

P = 128
KT = HIDDEN // P     # 56 k-tiles along hidden dim
NG = 4               # token groups per core
T_GRP = T_LOC // NG  # 512 tokens per group
TS_GRP = T_GRP // P  # 4 subtiles per group = 4 PSUM banks (2 groups in flight)
KC = 8               # k-tiles per DMA chunk (8KB/partition lines)
NKC = KT // KC       # 7 chunks

_NC_CACHE = {}


def build_gate_nc(t_loc=T_LOC, h=HIDDEN, e=NUM_EXPERTS, repeat=1):
    import concourse.mybir as mybir
    import concourse.tile as tile
    from concourse import bacc

    f32 = mybir.dt.float32
    fp16 = mybir.dt.float16

    nc = bacc.Bacc("TRN2", target_bir_lowering=False)
    # [p][group][ko][t]: per-partition contiguous KC*512 fp16 = 8KB chunks
    hsT_hi = nc.dram_tensor("hsT_hi", [P, NG * KT * T_GRP], fp16, kind="ExternalInput")
    hsT_lo = nc.dram_tensor("hsT_lo", [P, NG * KT * T_GRP], fp16, kind="ExternalInput")
    # [p][ko][0:256]=fp16(64*wT), [p][ko][256:512]=fp16((64*wT - hi) * 2^11)
    wT_cat = nc.dram_tensor("wT_cat", [P, KT * 2 * e], fp16, kind="ExternalInput")
    # outputs leave in on-chip staging order [p][ts][k]; host reorders to
    # [ts*128+p, k] (free) so the DMA moves one contiguous 512B line per
    # partition instead of 2048 x 32B descriptors
    idx_out = nc.dram_tensor(
        "topk_idx", [P, (t_loc // P) * TOP_K], mybir.dt.int32, kind="ExternalOutput"
    )
    w_out = nc.dram_tensor("topk_w", [P, (t_loc // P) * TOP_K], f32, kind="ExternalOutput")

    TS = t_loc // P  # 16 subtiles total

    with tile.TileContext(nc) as tc:
        with (
            tc.tile_pool(name="wpool", bufs=1) as wpool,
            tc.tile_pool(name="hpool", bufs=6) as hpool,
            tc.tile_pool(name="lpool", bufs=3) as lpool,
            tc.tile_pool(name="spool", bufs=4) as spool,
            tc.tile_pool(name="psum", bufs=8, space="PSUM") as psum_pool,
        ):
            # output staging: results accumulate here and leave as two large
            # descriptor DMAs at the end
            stage_idx = wpool.tile([P, TS, TOP_K], mybir.dt.int32, tag="sidx")
            stage_wv = wpool.tile([P, TS, TOP_K], f32, tag="swv")
            # gate weight: resident in SBUF, loaded in NKC chunks on the
            # otherwise-idle gpsimd (SWDGE) queue so the first matmul waits
            # on ~1MB, not 7.3MB.  (Routing early weight chunks via SP/Act
            # instead was tried and costs 70us/device: it delays the hs
            # stream those queues carry.)
            wt = wpool.tile([P, KT, 2 * e], fp16, tag="wt")
            wt_view = wT_cat[:, :].rearrange("p (ko e) -> p ko e", ko=KT)
            for kc in range(NKC):
                kslc = slice(kc * KC, (kc + 1) * KC)
                nc.gpsimd.dma_start(wt[:, kslc, :], wt_view[:, kslc, :])
            for rep in range(repeat):
                for grp in range(NG):
                    pts = []
                    for ts_i in range(TS_GRP):
                        pts.append(
                            psum_pool.tile(
                                [P, 2 * e], f32, tag="pt",
                                name=f"pt{rep}_{grp}_{ts_i}",
                            )
                        )
                    for kc in range(NKC):
                        base = grp * (KT * T_GRP) + kc * (KC * T_GRP)
                        hhi = hpool.tile(
                            [P, KC, T_GRP], fp16, tag="hs",
                            name=f"hshi{rep}_{grp}_{kc}",
                        )
                        nc.sync.dma_start(
                            hhi,
                            hsT_hi[:, base : base + KC * T_GRP].rearrange(
                                "p (ko t) -> p ko t", ko=KC
                            ),
                        )
                        hlo = hpool.tile(
                            [P, KC, T_GRP], fp16, tag="hs",
                            name=f"hslo{rep}_{grp}_{kc}",
                        )
                        nc.scalar.dma_start(
                            hlo,
                            hsT_lo[:, base : base + KC * T_GRP].rearrange(
                                "p (ko t) -> p ko t", ko=KC
                            ),
                        )
                        # last chunk runs ts-outer so bank ts_i finishes its
                        # full K-reduction ~2.7us before bank ts_i+1: the
                        # group's first three evacs overlap the remaining
                        # matmuls and only the final evac is tail-exposed
                        if kc == NKC - 1:
                            order = [
                                (ki, ts_i)
                                for ts_i in range(TS_GRP)
                                for ki in range(KC)
                            ]
                        else:
                            order = [
                                (ki, ts_i)
                                for ki in range(KC)
                                for ts_i in range(TS_GRP)
                            ]
                        for ki, ts_i in order:
                            k = kc * KC + ki
                            pt = pts[ts_i]
                            tslc = slice(ts_i * P, (ts_i + 1) * P)
                            # psum[:, 0:2e] += hshi . [whi | wlo]
                            nc.tensor.matmul(
                                pt,
                                hhi[:, ki, tslc],
                                wt[:, k, :],
                                start=(k == 0),
                                stop=False,
                            )
                            # psum[:, e:2e] += hslo . whi  (2^11 scale)
                            nc.tensor.matmul(
                                pt[:, e:],
                                hlo[:, ki, tslc],
                                wt[:, k, :e],
                                start=False,
                                stop=(k == KT - 1),
                            )
                    for ts_i in range(TS_GRP):
                        pt = pts[ts_i]
                        g_ts = grp * TS_GRP + ts_i
                        # work on 64x-scaled logits: top-k selection is
                        # monotonic under the scale, and the 1/64 folds into
                        # the Exp activation below — saves one DVE pass
                        # logits64 = psum_hi + 2^-11 * psum_cross
                        cross = lpool.tile([P, e], f32, tag="cross")
                        nc.vector.tensor_scalar_mul(cross, pt[:, e:], 1.0 / 2048.0)
                        logits = lpool.tile([P, e], f32, tag="logits")
                        nc.vector.tensor_add(logits, pt[:, :e], cross)
                        mx = spool.tile([P, TOP_K], f32, tag="mx")
                        nc.vector.max(out=mx, in_=logits)
                        idx_u = spool.tile([P, TOP_K], mybir.dt.uint32, tag="idxu")
                        nc.vector.max_index(idx_u, mx, logits)
                        nc.vector.tensor_copy(stage_idx[:, g_ts, :], idx_u)
                        # normalized top-k softmax: exp((v - v_max)/64) / sum
                        nm = spool.tile([P, 1], f32, tag="nm")
                        nc.vector.tensor_scalar_mul(nm, mx[:, 0:1], -1.0 / 64.0)
                        ev = spool.tile([P, TOP_K], f32, tag="ev")
                        sm = spool.tile([P, 1], f32, tag="sm")
                        nc.scalar.activation(
                            ev,
                            mx,
                            mybir.ActivationFunctionType.Exp,
                            bias=nm,
                            scale=1.0 / 64.0,
                            accum_out=sm,
                        )
                        rc = spool.tile([P, 1], f32, tag="rc")
                        nc.vector.reciprocal(rc, sm)
                        nc.vector.tensor_scalar_mul(stage_wv[:, g_ts, :], ev, rc)
            # groups 0-2's staged results are final before group 3 ends:
            # stream them out early so only the last group's small transfer
            # sits in the tail
            idx_view = idx_out[:, :].rearrange("p (ts k) -> p ts k", k=TOP_K)
            wv_view = w_out[:, :].rearrange("p (ts k) -> p ts k", k=TOP_K)
            s0 = (NG - 1) * TS_GRP
            nc.sync.dma_start(idx_view[:, :s0, :], stage_idx[:, :s0, :])
            nc.scalar.dma_start(wv_view[:, :s0, :], stage_wv[:, :s0, :])
            nc.sync.dma_start(idx_view[:, s0:, :], stage_idx[:, s0:, :])
            nc.scalar.dma_start(wv_view[:, s0:, :], stage_wv[:, s0:, :])
    nc.compile()
    return nc


def _get_nc():
    key = (T_LOC, HIDDEN, NUM_EXPERTS)
    if key not in _NC_CACHE:
        _NC_CACHE[key] = build_gate_nc(*key)
    return _NC_CACHE[key]


def _split_fp16(x, pre_scale=1.0):
    """x (f32) -> (hi, lo) fp16 with hi + lo/2^11 ~= pre_scale*x."""
    xs = x * np.float32(pre_scale) if pre_scale != 1.0 else x
    hi = xs.astype(np.float16)
    lo = ((xs - hi.astype(np.float32)) * np.float32(LO_SCALE)).astype(np.float16)
    return hi, lo


def _pack_hs(hs_part):
    """[t_loc, H] f32 -> [128, 4*56*512] fp16 hi/lo in [p][group][ko][t] order."""
    hsT = np.ascontiguousarray(hs_part.T)  # [H, t_loc]
    hi, lo = _split_fp16(hsT)
    out = []
    for x in (hi, lo):
        x4 = x.reshape(KT, P, NG, T_GRP)          # [ko][p][group][t]
        x4 = np.ascontiguousarray(x4.transpose(1, 2, 0, 3))  # [p][group][ko][t]
        out.append(x4.reshape(P, NG * KT * T_GRP))
    return out


def _prep_inputs(hs, w):
    wT = np.ascontiguousarray(w.T)  # [H, E]
    w_hi, w_lo = _split_fp16(wT, W_SCALE)
    w_cat = np.concatenate([w_hi, w_lo], axis=1)          # [H, 2E]
    w3 = w_cat.reshape(KT, P, 2 * NUM_EXPERTS)            # [ko][p][2e]
    w3 = np.ascontiguousarray(w3.transpose(1, 0, 2))      # [p][ko][2e]
    wT_packed = w3.reshape(P, KT * 2 * NUM_EXPERTS)
    in_maps = []
    for c in range(N_CORES):
        hs_hi, hs_lo = _pack_hs(hs[c * T_LOC : (c + 1) * T_LOC])
        in_maps.append({"hsT_hi": hs_hi, "hsT_lo": hs_lo, "wT_cat": wT_packed})
    return in_maps


_FN_CACHE = {}


def _make_runner(nc):
    """Compile a reusable 8-core PJRT callable (same lowering path as
    run_bass_kernel_spmd under axon, but cached so repeat kernel() calls
    skip re-tracing/compiling)."""
    import jax
    import concourse.mybir as mybir
    from concourse import bass2jax
    from jax.sharding import Mesh, NamedSharding, PartitionSpec
    from jax.experimental.shard_map import shard_map

    bass2jax.install_neuronx_cc_hook()
    partition_name = nc.partition_id_tensor.name if nc.partition_id_tensor else None
    in_names, out_names, out_avals, zero_shapes = [], [], [], []
    for alloc in nc.m.functions[0].allocations:
        if not isinstance(alloc, mybir.MemoryLocationSet):
            continue
        name = alloc.memorylocations[0].name
        if alloc.kind == "ExternalInput":
            if name != partition_name:
                in_names.append(name)
        elif alloc.kind == "ExternalOutput":
            shape = tuple(alloc.tensor_shape)
            dtype = mybir.dt.np(alloc.dtype)
            out_names.append(name)
            out_avals.append(jax.core.ShapedArray(shape, dtype))
            zero_shapes.append((shape, dtype))
    n_params = len(in_names)
    n_outs = len(out_avals)
    all_in_names = list(in_names) + list(out_names)
    if partition_name is not None:
        all_in_names.append(partition_name)

    def _body(*args):
        operands = list(args)
        if partition_name is not None:
            operands.append(bass2jax.partition_id_tensor())
        outs = bass2jax._bass_exec_p.bind(
            *operands,
            out_avals=tuple(out_avals),
            in_names=tuple(all_in_names),
            out_names=tuple(out_names),
            lowering_input_output_aliases=(),
            sim_require_finite=True,
            sim_require_nnan=True,
            nc=nc,
        )
        return tuple(outs)

    devices = jax.devices()[:N_CORES]
    mesh = Mesh(np.asarray(devices), ("core",))
    in_specs = (PartitionSpec("core"),) * (n_params + n_outs)
    out_specs = (PartitionSpec("core"),) * len(out_names)
    donate = tuple(range(n_params, n_params + n_outs))
    fn = jax.jit(
        shard_map(
            _body, mesh=mesh, in_specs=in_specs, out_specs=out_specs, check_rep=False
        ),
        donate_argnums=donate,
        keep_unused=True,
    )
    sharding = NamedSharding(mesh, PartitionSpec("core"))

    def run(in_maps):
        concat_in = [
            np.concatenate(
                [np.asarray(in_maps[c][nm]) for c in range(N_CORES)], axis=0
            )
            for nm in in_names
        ]
        zeros = [
            np.zeros((N_CORES * s[0], *s[1:]), dt) for s, dt in zero_shapes
        ]
        dev_in = [jax.device_put(x, sharding) for x in concat_in]
        out_arrs = fn(*dev_in, *zeros)
        return [
            {
                nm: np.asarray(out_arrs[i]).reshape(
                    N_CORES, *out_avals[i].shape
                )[c]
                for i, nm in enumerate(out_names)
            }
            for c in range(N_CORES)
        ]

    return run


def kernel(hidden_states, weight):
    hs = np.asarray(hidden_states, dtype=np.float32)
    w = np.asarray(weight, dtype=np.float32)
    assert hs.shape == (NUM_TOKENS, HIDDEN), hs.shape
    assert w.shape == (NUM_EXPERTS, HIDDEN), w.shape

    in_maps = _prep_inputs(hs, w)
    nc = _get_nc()
    try:
        if "run" not in _FN_CACHE:
            _FN_CACHE["run"] = _make_runner(nc)
        results = _FN_CACHE["run"](in_maps)
    except Exception:
        # fall back to the stock path if the cached-runner path breaks
        from concourse.bass_utils import run_bass_kernel_spmd

        results = run_bass_kernel_spmd(
            nc, in_maps, core_ids=list(range(N_CORES))
        ).results

    def _unstage(x):
        # [P, TS*K] staging order -> [t_loc, K] (token = ts*128 + p)
        ts_n = T_LOC // P
        return x.reshape(P, ts_n, TOP_K).transpose(1, 0, 2).reshape(T_LOC, TOP_K)

    topk_idx = np.concatenate([_unstage(r["topk_idx"]) for r in results], axis=0)
    topk_w = np.concatenate([_unstage(r["topk_w"]) for r in results], axis=0)
    row_idx = (
        np.arange(NUM_TOKENS * TOP_K, dtype=np.int32).reshape(TOP_K, NUM_TOKENS).T
    )
    return (
        topk_idx.astype(np.int32),
        topk_w.astype(np.float32),
        row_idx,
    )
